# revision 1
# baseline (speedup 1.0000x reference)
"""Multi-head causal attention with RoPE on 8 Trainium2 NeuronCores.

Sharding: 2 (batch) x 4 (head-groups of 4 heads). Each core computes
QKV projections, RoPE, flash-style causal attention and its slice of the
output projection for one batch and 4 heads; partial outputs are summed
on the host (row-sharded out_proj => partial-sum reduction).

Device layout choices (everything host-prepped to avoid on-device
transposes, fp32 has no DMA-transpose path):
  - x is passed pre-transposed per batch: xT [D, S] bf16
  - Q^T, K^T computed as [head_dim, S] (lhsT = W tile, rhs = xT)
  - V computed natural [S, head_dim] (lhsT = xT tile, rhs = Wv)
  - scores computed transposed [k, q]; softmax sum over k (partitions)
    via a full-width all-ones stationary matmul, which lands the same sum
    on every PSUM partition so normalization needs no broadcast
  - RoPE rotate-half done with a signed 128x128 permutation matmul
  - attention g-outer with per-q-group out-proj interleaved; scores
    pipelined one k-tile ahead of the PV/sum matmuls
"""

import math
import sys

import numpy as np

try:
    import concourse.bass as bass  # noqa: F401
except Exception:
    sys.path.insert(0, "/opt/trn_rl_repo")

import ml_dtypes

P = 128
B = 2
S = 2048
D = 2048
H = 16
HEAD = 128
N_CORES = 8
HG = 4            # head groups (tensor-parallel dimension)
HPG = H // HG     # heads per group = 4
DG = HPG * HEAD   # group width = 512
SG = 512          # q-group (free dim) size
DOUT = 2048

BF16 = ml_dtypes.bfloat16


def _emit(tc, io, cfg, sfx=""):
    """Emit the per-core program. io: dict of dram APs. cfg: sizes."""
    import concourse.mybir as mybir

    nc = tc.nc
    bf = mybir.dt.bfloat16
    f32 = mybir.dt.float32
    Exp = mybir.ActivationFunctionType.Exp

    s = cfg["S"]
    d = cfg["D"]
    dout = cfg["DOUT"]
    di_t = d // P          # d_in k-tiles
    st = s // P            # seq 128-tiles
    nsg = s // SG          # seq 512-groups
    nos = dout // SG       # out column slices
    inv_sqrt_hd = 1.0 / math.sqrt(HEAD)

    xT = io["xT"].rearrange("(o p) s -> p o s", p=P)
    wq = io["wq"].rearrange("(o p) n -> p o n", p=P)
    wk = io["wk"].rearrange("(o p) n -> p o n", p=P)
    wv = io["wv"].rearrange("(o p) n -> p o n", p=P)
    wo = io["wo"].rearrange("(o p) n -> p o n", p=P)

    const = tc.alloc_tile_pool(name="const" + sfx, bufs=1)
    stores = tc.alloc_tile_pool(name="stores" + sfx, bufs=1)
    ps_main = tc.alloc_tile_pool(name="ps_main" + sfx, bufs=3, space="PSUM")
    ps2 = tc.alloc_tile_pool(name="ps2" + sfx, bufs=2, space="PSUM")
    ps_sum = tc.alloc_tile_pool(name="ps_sum" + sfx, bufs=1, space="PSUM")

    # ---- constants (tiles only; DMAs emitted after the xT stream) ----
    cos_sb = const.tile([P, s], bf, tag="cos")
    sin_sb = const.tile([P, s], bf, tag="sin")
    rot_sb = const.tile([P, P], bf, tag="rot")
    mask_sb = const.tile([P, HG, SG], bf, tag="mask")
    ones_bf_sb = const.tile([P, P], bf, tag="ones_bf")
    wv_sb = const.tile([P, di_t, DG], bf, tag="wv")
    wo_sb = const.tile([P, HPG, dout], bf, tag="wo")

    # persistent activation stores
    qt_sb = stores.tile([P, HPG, s], bf, tag="qt")
    kt_sb = stores.tile([P, HPG, s], bf, tag="kt")
    v_sb = stores.tile([P, st, DG], bf, tag="v")
    ctx_sb = stores.tile([P, HPG, s], bf, tag="ctx")

    # ---- phase 1: projections + RoPE ----
    with tc.tile_pool(name="xt" + sfx, bufs=1) as xtp, \
         tc.tile_pool(name="wqk" + sfx, bufs=2) as wqkp, \
         tc.tile_pool(name="p1tmp" + sfx, bufs=4) as p1tmp:
        xt_sb = xtp.tile([P, di_t, s], bf, tag="xt")
        # wv first (V needs it), then xT by column-group so early V/QK tiles
        # land fast; remaining constants follow.
        for o in range(di_t):
            nc.sync.dma_start(wv_sb[:, o, :], wv[:, o, :])
            nc.sync.dma_start(xt_sb[:, o, 0:SG], xT[:, o, 0:SG])
        for g in range(1, nsg):
            for o in range(di_t):
                nc.sync.dma_start(
                    xt_sb[:, o, g * SG:(g + 1) * SG], xT[:, o, g * SG:(g + 1) * SG]
                )
        nc.sync.dma_start(cos_sb[:], io["cosT"][:])
        nc.sync.dma_start(sin_sb[:], io["sinT"][:])
        nc.sync.dma_start(rot_sb[:], io["rot"][:])
        nc.sync.dma_start(mask_sb[:], io["masks"][:])
        nc.sync.dma_start(ones_bf_sb[:], io["ones_bf"][:])
        for o in range(HPG):
            nc.sync.dma_start(wo_sb[:, o, :], wo[:, o, :])

        # V natural layout: [s_tile, DG]
        for si in range(st):
            pv = ps_main.tile([P, SG], f32, tag="ps")
            for o in range(di_t):
                nc.tensor.matmul(
                    pv[:, :DG],
                    lhsT=xt_sb[:, o, si * P:(si + 1) * P],
                    rhs=wv_sb[:, o, :],
                    start=(o == 0),
                    stop=(o == di_t - 1),
                )
            nc.vector.tensor_copy(v_sb[:, si, :], pv[:, :DG])

        # Q^T, K^T with RoPE, per head; rot-MM pipelined behind the next
        # projection block so the PE never waits on the ACT psum->sbuf copy
        for h in range(HPG):
            wq_t = wqkp.tile([P, di_t, P], bf, tag="wq")
            wk_t = wqkp.tile([P, di_t, P], bf, tag="wk")
            for o in range(di_t):
                nc.sync.dma_start(wq_t[:, o, :], wq[:, o, h * P:(h + 1) * P])
                nc.sync.dma_start(wk_t[:, o, :], wk[:, o, h * P:(h + 1) * P])

            def emit_rope(qa, dst, hh, sl):
                pr = ps_main.tile([P, SG], f32, tag="ps")
                nc.tensor.matmul(pr, lhsT=rot_sb, rhs=qa, start=True, stop=True)
                t1 = p1tmp.tile([P, SG], bf, tag="t1")
                nc.vector.tensor_mul(t1, qa, cos_sb[:, sl])
                t2 = p1tmp.tile([P, SG], bf, tag="t2")
                nc.vector.tensor_mul(t2, pr, sin_sb[:, sl])
                nc.vector.tensor_add(dst[:, hh, sl], t1, t2)

            pending = []
            for g in range(nsg):
                sl = slice(g * SG, (g + 1) * SG)
                for w_t, dst in ((wq_t, qt_sb), (wk_t, kt_sb)):
                    pq = ps_main.tile([P, SG], f32, tag="ps")
                    for o in range(di_t):
                        nc.tensor.matmul(
                            pq,
                            lhsT=w_t[:, o, :],
                            rhs=xt_sb[:, o, sl],
                            start=(o == 0),
                            stop=(o == di_t - 1),
                        )
                    qa = p1tmp.tile([P, SG], bf, tag="qa")
                    nc.scalar.copy(qa, pq)
                    pending.append((qa, dst, h, sl))
                    while len(pending) > 2:
                        emit_rope(*pending.pop(0))
            while pending:
                emit_rope(*pending.pop(0))

    # ---- phase 2+3: attention interleaved with output projection ----
    # g outer so each q-group's out-proj tiles become ready early and fill
    # the PE while later q-groups' softmax runs. Scores pipelined one k-tile
    # ahead of PV; softmax sum uses a full-width ones stationary so the
    # normalization needs no cross-partition broadcast.
    with tc.tile_pool(name="p2tmp" + sfx, bufs=10) as p2tmp, \
         tc.tile_pool(name="p2rb" + sfx, bufs=3) as p2rb, \
         tc.tile_pool(name="outp" + sfx, bufs=3) as outp:
        for g in range(nsg):
            qsl = slice(g * SG, (g + 1) * SG)
            jmax = min((g + 1) * SG // P, st)
            for h in range(HPG):
                pctx = ps_main.tile([P, SG], f32, tag="ps")
                psum_l = ps_sum.tile([P, SG], f32, tag="l")

                # stream 1: paired score MMs + one exp per [P, 2*SG];
                # DVE pre-reduces each pair so the softmax-sum matmul
                # stream is halved
                ats = []
                dsums = []
                for j in range(0, jmax, 2):
                    ps2t = ps2.tile([P, 2, SG], f32, tag="ps2")
                    nc.tensor.matmul(
                        ps2t[:, 0, :],
                        lhsT=kt_sb[:, h, j * P:(j + 1) * P],
                        rhs=qt_sb[:, h, qsl],
                        start=True,
                        stop=True,
                    )
                    nc.tensor.matmul(
                        ps2t[:, 1, :],
                        lhsT=kt_sb[:, h, (j + 1) * P:(j + 2) * P],
                        rhs=qt_sb[:, h, qsl],
                        start=True,
                        stop=True,
                    )
                    at2 = p2tmp.tile([P, 2, SG], bf, tag="at")
                    nc.scalar.activation(at2, ps2t, Exp, scale=inv_sqrt_hd)
                    r = j - (g * SG // P)
                    if r >= 0:
                        nc.vector.tensor_mul(at2, at2, mask_sb[:, r:r + 2, :])
                    ats.append(at2)
                    dsum = p2tmp.tile([P, SG], bf, tag="dsum")
                    nc.vector.tensor_add(dsum, at2[:, 0, :], at2[:, 1, :])
                    dsums.append(dsum)

                # stream 2: PV accumulation (wait-free after exps drain)
                for idx, at2 in enumerate(ats):
                    for jj in range(2):
                        j = 2 * idx + jj
                        nc.tensor.matmul(
                            pctx,
                            lhsT=v_sb[:, j, h * P:(h + 1) * P],
                            rhs=at2[:, jj, :],
                            start=(j == 0),
                            stop=(j == jmax - 1),
                        )
                # second DVE tree level: combine pair-sums, then one
                # softmax-sum matmul per four k-tiles
                dsums2 = []
                for i in range(0, len(dsums), 2):
                    if i + 1 < len(dsums):
                        d2 = p2tmp.tile([P, SG], bf, tag="dsum2")
                        nc.vector.tensor_add(d2, dsums[i], dsums[i + 1])
                        dsums2.append(d2)
                    else:
                        dsums2.append(dsums[i])
                for idx, dsum in enumerate(dsums2):
                    nc.tensor.matmul(
                        psum_l,
                        lhsT=ones_bf_sb[:],
                        rhs=dsum,
                        start=(idx == 0),
                        stop=(idx == len(dsums2) - 1),
                    )
                rec = p2rb.tile([P, SG], f32, tag="rec")
                nc.vector.reciprocal_approx_fast(rec, psum_l)
                nc.vector.tensor_mul(ctx_sb[:, h, qsl], pctx, rec)

            for qt in range(4 * g, 4 * (g + 1)):
                for dsl in range(nos):
                    po = ps_main.tile([P, SG], f32, tag="ps")
                    for h in range(HPG):
                        nc.tensor.matmul(
                            po,
                            lhsT=ctx_sb[:, h, qt * P:(qt + 1) * P],
                            rhs=wo_sb[:, h, dsl * SG:(dsl + 1) * SG],
                            start=(h == 0),
                            stop=(h == HPG - 1),
                        )
                    ob = outp.tile([P, SG], f32, tag="ob")
                    nc.vector.tensor_copy(ob, po)
                    nc.sync.dma_start(
                        io["out"][qt * P:(qt + 1) * P, dsl * SG:(dsl + 1) * SG], ob
                    )

    for pool in (ps_sum, ps2, ps_main, stores, const):
        pool.release()


def build_program(cfg=None):
    import concourse.bacc as bacc
    import concourse.mybir as mybir
    import concourse.tile as tile

    cfg = cfg or {"S": S, "D": D, "DOUT": DOUT}
    bf = mybir.dt.bfloat16
    f32 = mybir.dt.float32
    nc = bacc.Bacc()
    io = {
        "xT": nc.dram_tensor("xT", [cfg["D"], cfg["S"]], bf, kind="ExternalInput"),
        "wq": nc.dram_tensor("wq", [cfg["D"], DG], bf, kind="ExternalInput"),
        "wk": nc.dram_tensor("wk", [cfg["D"], DG], bf, kind="ExternalInput"),
        "wv": nc.dram_tensor("wv", [cfg["D"], DG], bf, kind="ExternalInput"),
        "wo": nc.dram_tensor("wo", [DG, cfg["DOUT"]], bf, kind="ExternalInput"),
        "cosT": nc.dram_tensor("cosT", [P, cfg["S"]], bf, kind="ExternalInput"),
        "sinT": nc.dram_tensor("sinT", [P, cfg["S"]], bf, kind="ExternalInput"),
        "rot": nc.dram_tensor("rot", [P, P], bf, kind="ExternalInput"),
        "masks": nc.dram_tensor("masks", [P, HG, SG], bf, kind="ExternalInput"),
        "ones_bf": nc.dram_tensor("ones_bf", [P, P], bf, kind="ExternalInput"),
        "ones_f": nc.dram_tensor("ones_f", [1, P], f32, kind="ExternalInput"),
        "out": nc.dram_tensor(
            "out", [cfg["S"], cfg["DOUT"]], f32, kind="ExternalOutput"
        ),
    }
    with tile.TileContext(nc) as tc:
        for rep in range(cfg.get("repeat", 1)):
            _emit(tc, io, cfg, sfx=f"_r{rep}")
    nc.finalize()
    return nc


def host_constants(s=S):
    inv = 1.0 / (10000.0 ** (np.arange(0, HEAD, 2, dtype=np.float32) / HEAD))
    pos = np.arange(s, dtype=np.float32)
    ang = pos[:, None] * inv[None, :]
    ang = np.concatenate([ang, ang], axis=-1)          # (s, HEAD)
    cosT = np.cos(ang).T.astype(BF16).copy()           # (HEAD, s)
    sinT = np.sin(ang).T.astype(BF16).copy()
    rot = np.zeros((HEAD, HEAD), np.float32)
    for dd in range(64):
        rot[dd, dd + 64] = -1.0
        rot[dd + 64, dd] = 1.0
    rotT = rot.T.astype(BF16).copy()                   # lhsT for out = rot @ q
    kk = np.arange(P)[:, None, None]
    rr = np.arange(HG)[None, :, None]
    qq = np.arange(SG)[None, None, :]
    masks = (kk <= qq - P * rr).astype(BF16)           # (P, HG, SG)
    ones_bf = np.ones((P, P), BF16)
    ones_f = np.ones((1, P), np.float32)
    return cosT, sinT, rotT, masks, ones_bf, ones_f


def kernel(x, W_query, W_key, W_value, W_out):
    from concourse.bass_utils import run_bass_kernel_spmd

    x = np.asarray(x)
    in_dtype = x.dtype
    nc = build_program()
    cosT, sinT, rotT, masks, ones_bf, ones_f = host_constants()

    xTb = [np.ascontiguousarray(np.asarray(x[b]).T).astype(BF16) for b in range(B)]
    in_maps = []
    for core in range(N_CORES):
        b, g = divmod(core, HG)
        gsl = slice(g * DG, (g + 1) * DG)
        in_maps.append({
            "xT": xTb[b],
            "wq": np.asarray(W_query)[:, gsl].astype(BF16).copy(),
            "wk": np.asarray(W_key)[:, gsl].astype(BF16).copy(),
            "wv": np.asarray(W_value)[:, gsl].astype(BF16).copy(),
            "wo": np.asarray(W_out)[gsl, :].astype(BF16).copy(),
            "cosT": cosT, "sinT": sinT, "rot": rotT, "masks": masks,
            "ones_bf": ones_bf, "ones_f": ones_f,
        })

    res = run_bass_kernel_spmd(nc, in_maps, core_ids=list(range(N_CORES)))
    out = np.zeros((B, S, DOUT), np.float32)
    for core in range(N_CORES):
        b = core // HG
        out[b] += res.results[core]["out"]
    return out.astype(in_dtype, copy=False)



# revision 15
# speedup vs baseline: 1.1291x; 1.1291x over previous
"""Multi-head causal attention with RoPE on 8 Trainium2 NeuronCores.

Sharding: 2 (batch) x 4 (head-groups of 4 heads). Each core computes
QKV projections, RoPE, flash-style causal attention and its slice of the
output projection for one batch and 4 heads; partial outputs are summed
on the host (row-sharded out_proj => partial-sum reduction).

Device layout choices (everything host-prepped to avoid on-device
transposes, fp32 has no DMA-transpose path):
  - x is passed pre-transposed per batch: xT [D, S] bf16
  - Q^T, K^T computed as [head_dim, S] (lhsT = W tile, rhs = xT)
  - V computed natural [S, head_dim] (lhsT = xT tile, rhs = Wv);
    emitted o-major across 4 parallel si-chains so the PE tracks the
    x DMA stream during startup (no long cold-start stall)
  - scores computed transposed [k, q]; softmax sum over k (partitions)
    via gpsimd partition_all_reduce (lands the sum on every partition,
    so normalization needs no broadcast and no PE ones-matmul)
  - RoPE rotate-half via two SBUF->SBUF partition-swap DMAs plus a
    sign-folded sin constant (no PE permutation matmul)
  - causal diagonal k-tiles use width-trimmed score/PV matmuls
    (only the q >= k columns), one 128x128 tril mask for the true
    diagonal block
  - attention g-outer with per-q-group out-proj interleaved
"""

import math
import sys

import numpy as np

try:
    import concourse.bass as bass  # noqa: F401
except Exception:
    sys.path.insert(0, "/opt/trn_rl_repo")

import ml_dtypes

P = 128
B = 2
S = 2048
D = 2048
H = 16
HEAD = 128
N_CORES = 8
HG = 4            # head groups (tensor-parallel dimension)
HPG = H // HG     # heads per group = 4
DG = HPG * HEAD   # group width = 512
SG = 512          # q-group (free dim) size
DOUT = 2048

BF16 = ml_dtypes.bfloat16


def _emit(tc, io, cfg, sfx=""):
    """Emit the per-core program. io: dict of dram APs. cfg: sizes."""
    import concourse.mybir as mybir
    nc = tc.nc
    bf = mybir.dt.bfloat16
    f32 = mybir.dt.float32
    Exp = mybir.ActivationFunctionType.Exp

    s = cfg["S"]
    d = cfg["D"]
    dout = cfg["DOUT"]
    di_t = d // P          # d_in k-tiles
    st = s // P            # seq 128-tiles
    nsg = s // SG          # seq 512-groups
    nos = dout // SG       # out column slices
    inv_sqrt_hd = 1.0 / math.sqrt(HEAD)

    xT = io["xT"].rearrange("(o p) s -> p o s", p=P)
    wq = io["wq"].rearrange("(o p) n -> p o n", p=P)
    wk = io["wk"].rearrange("(o p) n -> p o n", p=P)
    wv = io["wv"].rearrange("(o p) n -> p o n", p=P)
    wo = io["wo"].rearrange("(o p) n -> p o n", p=P)

    const = tc.alloc_tile_pool(name="const" + sfx, bufs=1)
    stores = tc.alloc_tile_pool(name="stores" + sfx, bufs=1)

    # ---- constants (tiles only; DMAs emitted inside the phase-1 stream) ----
    cos_sb = const.tile([P, s], bf, tag="cos")
    sin_sb = const.tile([P, s], bf, tag="sin")     # sign-folded rope sin
    mask_sb = const.tile([P, P], bf, tag="mask")   # tril (k<=q) diagonal mask
    ones_sb = const.tile([P, P], bf, tag="ones")
    wv_sb = const.tile([P, di_t, DG], bf, tag="wv")
    wo_sb = const.tile([P, HPG, dout], bf, tag="wo")

    # persistent activation stores
    qt_sb = stores.tile([P, HPG, s], bf, tag="qt")
    kt_sb = stores.tile([P, HPG, s], bf, tag="kt")
    v_sb = stores.tile([P, st, DG], bf, tag="v")
    ctx_sb = stores.tile([P, HPG, s], bf, tag="ctx")

    # ---- phase 1: projections + RoPE ----
    with tc.tile_pool(name="xt" + sfx, bufs=1) as xtp, \
         tc.tile_pool(name="wqk" + sfx, bufs=2) as wqkp, \
         tc.tile_pool(name="p1tmp" + sfx, bufs=4) as p1tmp, \
         tc.tile_pool(name="p1sw" + sfx, bufs=4) as p1sw, \
         tc.tile_pool(name="ps_v" + sfx, bufs=4, space="PSUM") as ps_v, \
         tc.tile_pool(name="ps_qk" + sfx, bufs=3, space="PSUM") as ps_qk:
        xt_sb = xtp.tile([P, di_t, s], bf, tag="xt")
        # wv + the first xT column-group land first so V starts fast;
        # remaining constants and groups follow.
        for o in range(di_t):
            nc.sync.dma_start(wv_sb[:, o, :], wv[:, o, :])
            nc.sync.dma_start(xt_sb[:, o, 0:SG], xT[:, o, 0:SG])
        nc.sync.dma_start(cos_sb[:], io["cosT"][:])
        nc.sync.dma_start(sin_sb[:], io["sinT"][:])
        nc.sync.dma_start(mask_sb[:], io["mask"][:])
        nc.sync.dma_start(ones_sb[:], io["ones"][:])
        for g in range(1, nsg):
            for o in range(di_t):
                nc.sync.dma_start(
                    xt_sb[:, o, g * SG:(g + 1) * SG], xT[:, o, g * SG:(g + 1) * SG]
                )
        for o in range(HPG):
            nc.sync.dma_start(wo_sb[:, o, :], wo[:, o, :])

        # V natural layout [s_tile, DG]. First 4 si-chains run o-major in
        # parallel so early MMs only need the first xT slices (DMA-paced
        # startup); the rest run chain-at-a-time so evacuations overlap.
        pvs = [
            ps_v.tile([P, SG], f32, tag="psv", name=f"pv0_{i}")
            for i in range(4)
        ]
        for o in range(di_t):
            for q4 in range(4):
                nc.tensor.matmul(
                    pvs[q4][:, :DG],
                    lhsT=xt_sb[:, o, q4 * P:(q4 + 1) * P],
                    rhs=wv_sb[:, o, :],
                    start=(o == 0),
                    stop=(o == di_t - 1),
                )
        for q4 in range(4):
            nc.vector.tensor_copy(v_sb[:, q4, :], pvs[q4][:, :DG])
        for si in range(4, st):
            pv = ps_v.tile([P, SG], f32, tag="psv")
            for o in range(di_t):
                nc.tensor.matmul(
                    pv[:, :DG],
                    lhsT=xt_sb[:, o, si * P:(si + 1) * P],
                    rhs=wv_sb[:, o, :],
                    start=(o == 0),
                    stop=(o == di_t - 1),
                )
            nc.vector.tensor_copy(v_sb[:, si, :], pv[:, :DG])

        # Q^T, K^T with RoPE, per head. Rotate-half = partition swap via
        # two SBUF->SBUF DMAs + sign-folded sin; pipelined 2 deep so the
        # PE never waits on the ACT psum->sbuf copy or the swap DMA.
        def emit_rope(qa, qsw, dst, hh, sl):
            t1 = p1tmp.tile([P, SG], bf, tag="t1")
            nc.vector.tensor_mul(t1, qa, cos_sb[:, sl])
            t2 = p1tmp.tile([P, SG], bf, tag="t2")
            nc.vector.tensor_mul(t2, qsw, sin_sb[:, sl])
            nc.vector.tensor_add(dst[:, hh, sl], t1, t2)

        pending = []
        for h in range(HPG):
            wq_t = wqkp.tile([P, di_t, P], bf, tag="wq")
            wk_t = wqkp.tile([P, di_t, P], bf, tag="wk")
            for o in range(di_t):
                nc.sync.dma_start(wq_t[:, o, :], wq[:, o, h * P:(h + 1) * P])
                nc.sync.dma_start(wk_t[:, o, :], wk[:, o, h * P:(h + 1) * P])

            for g in range(nsg):
                sl = slice(g * SG, (g + 1) * SG)
                for w_t, dst in ((wq_t, qt_sb), (wk_t, kt_sb)):
                    pq = ps_qk.tile([P, SG], f32, tag="ps")
                    for o in range(di_t):
                        nc.tensor.matmul(
                            pq,
                            lhsT=w_t[:, o, :],
                            rhs=xt_sb[:, o, sl],
                            start=(o == 0),
                            stop=(o == di_t - 1),
                        )
                    qa = p1tmp.tile([P, SG], bf, tag="qa")
                    nc.scalar.copy(qa, pq)
                    qsw = p1sw.tile([P, SG], bf, tag="qsw")
                    nc.sync.dma_start(qsw[0:64, :], qa[64:128, :])
                    nc.sync.dma_start(qsw[64:128, :], qa[0:64, :])
                    pending.append((qa, qsw, dst, h, sl))
                    while len(pending) > 2:
                        emit_rope(*pending.pop(0))
        while pending:
            emit_rope(*pending.pop(0))

    # ---- phase 2+3: attention interleaved with output projection ----
    # g outer so each q-group's out-proj tiles become ready early and fill
    # the PE while later q-groups' softmax runs. Diagonal k-tiles use
    # width-trimmed matmuls; softmax sums pair-tree on DVE into one tile,
    # then a single full-width ones-matmul per (h, g) lands the sum on
    # every PSUM partition so normalization needs no broadcast.
    ps_sc = tc.alloc_tile_pool(name="ps_sc" + sfx, bufs=2, space="PSUM")
    ps_main = tc.alloc_tile_pool(name="ps_main" + sfx, bufs=3, space="PSUM")
    ps_l = tc.alloc_tile_pool(name="ps_l" + sfx, bufs=1, space="PSUM")
    with tc.tile_pool(name="p2tmp" + sfx, bufs=10) as p2tmp, \
         tc.tile_pool(name="p2rb" + sfx, bufs=3) as p2rb, \
         tc.tile_pool(name="outp" + sfx, bufs=3) as outp:
        for g in range(nsg):
            qsl = slice(g * SG, (g + 1) * SG)
            jf = 4 * g          # full (below-diagonal) k-tiles
            for h in range(HPG):
                pctx = ps_main.tile([P, SG], f32, tag="ps")

                # stream 1: score MMs + exp. Full tiles in pairs; the 4
                # diagonal tiles packed into two psum tiles with trimmed
                # widths (512,384 | 256,128); tails zeroed so the sum tree
                # is uniform.
                ats = []         # (tile, [(slot, qoff, w), ...])
                for j in range(0, jf, 2):
                    ps2t = ps_sc.tile([P, 2, SG], f32, tag="ps2")
                    for jj in range(2):
                        nc.tensor.matmul(
                            ps2t[:, jj, :],
                            lhsT=kt_sb[:, h, (j + jj) * P:(j + jj + 1) * P],
                            rhs=qt_sb[:, h, qsl],
                            start=True,
                            stop=True,
                        )
                    at2 = p2tmp.tile([P, 2, SG], bf, tag="at")
                    nc.scalar.activation(at2, ps2t, Exp, scale=inv_sqrt_hd)
                    ats.append((at2, [(0, 0, SG), (1, 0, SG)]))
                for pack in ((0, 1), (2, 3)):
                    psd = ps_sc.tile([P, 2, SG], f32, tag="ps2")
                    slots = []
                    for slot, r in enumerate(pack):
                        j = jf + r
                        qoff = r * P
                        w = SG - qoff
                        nc.tensor.matmul(
                            psd[:, slot, 0:w],
                            lhsT=kt_sb[:, h, j * P:(j + 1) * P],
                            rhs=qt_sb[:, h, g * SG + qoff:(g + 1) * SG],
                            start=True,
                            stop=True,
                        )
                        slots.append((slot, qoff, w))
                    dat = p2tmp.tile([P, 2, SG], bf, tag="at")
                    for slot, qoff, w in slots:
                        # write at the global q-offset so all tiles align
                        # column-wise; zero the front for the sum tree
                        nc.scalar.activation(
                            dat[:, slot, qoff:SG], psd[:, slot, 0:w], Exp,
                            scale=inv_sqrt_hd,
                        )
                        if qoff > 0:
                            nc.vector.memset(dat[:, slot, 0:qoff], 0)
                    ats.append((dat, slots))
                # mask the true diagonal 128-block of each trimmed tile
                for dat, slots in ats[-2:]:
                    for slot, qoff, w in slots:
                        nc.vector.tensor_mul(
                            dat[:, slot, qoff:qoff + P],
                            dat[:, slot, qoff:qoff + P], mask_sb
                        )

                # softmax sums (they only need the exps): DVE pair-tree
                # to one tile (diag tails are zeroed, so full-width adds),
                # then one ones-matmul -> every partition holds the sum.
                dsums = []
                for at2, _slots in ats:
                    dsv = p2tmp.tile([P, SG], bf, tag="ds")
                    nc.vector.tensor_add(dsv, at2[:, 0, :], at2[:, 1, :])
                    dsums.append(dsv)
                while len(dsums) > 1:
                    nxt = []
                    for i in range(0, len(dsums) - 1, 2):
                        d2 = p2tmp.tile([P, SG], bf, tag="ds2")
                        nc.vector.tensor_add(d2, dsums[i], dsums[i + 1])
                        nxt.append(d2)
                    if len(dsums) % 2:
                        nxt.append(dsums[-1])
                    dsums = nxt
                psum_l = ps_l.tile([P, SG], f32, tag="l")
                nc.tensor.matmul(
                    psum_l, lhsT=ones_sb, rhs=dsums[0], start=True, stop=True
                )
                rec = p2rb.tile([P, SG], f32, tag="rec")
                nc.vector.reciprocal_approx_fast(rec, psum_l)

                # PV accumulation in j (k-tile) order
                nmm = jf + 4
                mm_i = 0
                for at2, slots in ats:
                    for slot, qoff, w in slots:
                        nc.tensor.matmul(
                            pctx[:, qoff:SG],
                            lhsT=v_sb[:, mm_i, h * P:(h + 1) * P],
                            rhs=at2[:, slot, qoff:SG],
                            start=(mm_i == 0),
                            stop=(mm_i == nmm - 1),
                        )
                        mm_i += 1
                nc.vector.tensor_mul(ctx_sb[:, h, qsl], pctx, rec)

            for qt in range(4 * g, 4 * (g + 1)):
                for dsl in range(nos):
                    po = ps_main.tile([P, SG], f32, tag="ps")
                    for h in range(HPG):
                        nc.tensor.matmul(
                            po,
                            lhsT=ctx_sb[:, h, qt * P:(qt + 1) * P],
                            rhs=wo_sb[:, h, dsl * SG:(dsl + 1) * SG],
                            start=(h == 0),
                            stop=(h == HPG - 1),
                        )
                    ob = outp.tile([P, SG], f32, tag="ob")
                    nc.vector.tensor_copy(ob, po)
                    nc.sync.dma_start(
                        io["out"][qt * P:(qt + 1) * P, dsl * SG:(dsl + 1) * SG], ob
                    )

    for pool in (ps_l, ps_main, ps_sc, stores, const):
        pool.release()


def build_program(cfg=None):
    import concourse.bacc as bacc
    import concourse.mybir as mybir
    import concourse.tile as tile

    cfg = cfg or {"S": S, "D": D, "DOUT": DOUT}
    bf = mybir.dt.bfloat16
    f32 = mybir.dt.float32
    nc = bacc.Bacc()
    io = {
        "xT": nc.dram_tensor("xT", [cfg["D"], cfg["S"]], bf, kind="ExternalInput"),
        "wq": nc.dram_tensor("wq", [cfg["D"], DG], bf, kind="ExternalInput"),
        "wk": nc.dram_tensor("wk", [cfg["D"], DG], bf, kind="ExternalInput"),
        "wv": nc.dram_tensor("wv", [cfg["D"], DG], bf, kind="ExternalInput"),
        "wo": nc.dram_tensor("wo", [DG, cfg["DOUT"]], bf, kind="ExternalInput"),
        "cosT": nc.dram_tensor("cosT", [P, cfg["S"]], bf, kind="ExternalInput"),
        "sinT": nc.dram_tensor("sinT", [P, cfg["S"]], bf, kind="ExternalInput"),
        "mask": nc.dram_tensor("mask", [P, P], bf, kind="ExternalInput"),
        "ones": nc.dram_tensor("ones", [P, P], bf, kind="ExternalInput"),
        "out": nc.dram_tensor(
            "out", [cfg["S"], cfg["DOUT"]], f32, kind="ExternalOutput"
        ),
    }
    with tile.TileContext(nc) as tc:
        for rep in range(cfg.get("repeat", 1)):
            _emit(tc, io, cfg, sfx=f"_r{rep}")
    nc.finalize()
    return nc


def host_constants(s=S):
    inv = 1.0 / (10000.0 ** (np.arange(0, HEAD, 2, dtype=np.float32) / HEAD))
    pos = np.arange(s, dtype=np.float32)
    ang = pos[:, None] * inv[None, :]
    ang = np.concatenate([ang, ang], axis=-1)          # (s, HEAD)
    cosT = np.cos(ang).T.astype(BF16).copy()           # (HEAD, s)
    sinT = np.sin(ang).T.astype(np.float32)
    sinT[0:64, :] *= -1.0                              # sign-folded rotate-half
    sinT = sinT.astype(BF16).copy()
    kk = np.arange(P)[:, None]
    qq = np.arange(P)[None, :]
    mask = (kk <= qq).astype(BF16)                     # (P, P) tril in [k, q]
    ones = np.ones((P, P), BF16)
    return cosT, sinT, mask, ones


def build_in_maps(x, W_query, W_key, W_value, W_out):
    cosT, sinT, mask, ones = host_constants()
    xTb = [np.ascontiguousarray(np.asarray(x[b]).T).astype(BF16) for b in range(B)]
    in_maps = []
    for core in range(N_CORES):
        b, g = divmod(core, HG)
        gsl = slice(g * DG, (g + 1) * DG)
        in_maps.append({
            "xT": xTb[b],
            "wq": np.asarray(W_query)[:, gsl].astype(BF16).copy(),
            "wk": np.asarray(W_key)[:, gsl].astype(BF16).copy(),
            "wv": np.asarray(W_value)[:, gsl].astype(BF16).copy(),
            "wo": np.asarray(W_out)[gsl, :].astype(BF16).copy(),
            "cosT": cosT, "sinT": sinT, "mask": mask, "ones": ones,
        })
    return in_maps


def gather_out(results, in_dtype=np.float32):
    out = np.zeros((B, S, DOUT), np.float32)
    for core in range(N_CORES):
        out[core // HG] += results[core]["out"]
    return out.astype(in_dtype, copy=False)


def kernel(x, W_query, W_key, W_value, W_out):
    from concourse.bass_utils import run_bass_kernel_spmd

    x = np.asarray(x)
    nc = build_program()
    in_maps = build_in_maps(x, W_query, W_key, W_value, W_out)
    res = run_bass_kernel_spmd(nc, in_maps, core_ids=list(range(N_CORES)))
    return gather_out(res.results, x.dtype)


# revision 22
# speedup vs baseline: 1.1625x; 1.0296x over previous
"""Multi-head causal attention with RoPE on 8 Trainium2 NeuronCores.

Sharding: 2 (batch) x 4 (head-groups of 4 heads). Each core computes
QKV projections, RoPE, flash-style causal attention and its slice of the
output projection for one batch and 4 heads; partial outputs are summed
on the host (row-sharded out_proj => partial-sum reduction).

Device layout choices (everything host-prepped to avoid on-device
transposes, fp32 has no DMA-transpose path):
  - x is passed pre-transposed per batch: xT [D, S] bf16
  - Q^T, K^T computed as [head_dim, S] (lhsT = W tile, rhs = xT)
  - V computed natural [S, head_dim] (lhsT = xT tile, rhs = Wv);
    emitted o-major across 4 parallel si-chains so the PE tracks the
    x DMA stream during startup (no long cold-start stall)
  - scores computed transposed [k, q]; softmax sum over k (partitions)
    via gpsimd partition_all_reduce (lands the sum on every partition,
    so normalization needs no broadcast and no PE ones-matmul)
  - RoPE rotate-half via two SBUF->SBUF partition-swap DMAs plus a
    sign-folded sin constant (no PE permutation matmul)
  - causal diagonal k-tiles use width-trimmed score/PV matmuls
    (only the q >= k columns), one 128x128 tril mask for the true
    diagonal block
  - attention g-outer with per-q-group out-proj interleaved
"""

import math
import sys

import numpy as np

try:
    import concourse.bass as bass  # noqa: F401
except Exception:
    sys.path.insert(0, "/opt/trn_rl_repo")

import ml_dtypes

P = 128
B = 2
S = 2048
D = 2048
H = 16
HEAD = 128
N_CORES = 8
HG = 4            # head groups (tensor-parallel dimension)
HPG = H // HG     # heads per group = 4
DG = HPG * HEAD   # group width = 512
SG = 512          # q-group (free dim) size
DOUT = 2048

BF16 = ml_dtypes.bfloat16


def _emit(tc, io, cfg, sfx=""):
    """Emit the per-core program. io: dict of dram APs. cfg: sizes."""
    import concourse.mybir as mybir
    nc = tc.nc
    bf = mybir.dt.bfloat16
    f32 = mybir.dt.float32
    Exp = mybir.ActivationFunctionType.Exp

    s = cfg["S"]
    d = cfg["D"]
    dout = cfg["DOUT"]
    di_t = d // P          # d_in k-tiles
    st = s // P            # seq 128-tiles
    nsg = s // SG          # seq 512-groups
    nos = dout // SG       # out column slices
    inv_sqrt_hd = 1.0 / math.sqrt(HEAD)

    xT = io["xT"].rearrange("(o p) s -> p o s", p=P)
    wq = io["wq"].rearrange("(o p) n -> p o n", p=P)
    wk = io["wk"].rearrange("(o p) n -> p o n", p=P)
    wv = io["wv"].rearrange("(o p) n -> p o n", p=P)
    wo = io["wo"].rearrange("(o p) n -> p o n", p=P)

    const = tc.alloc_tile_pool(name="const" + sfx, bufs=1)
    stores = tc.alloc_tile_pool(name="stores" + sfx, bufs=1)

    # ---- constants (tiles only; DMAs emitted inside the phase-1 stream) ----
    cos_sb = const.tile([P, s], bf, tag="cos")
    sin_sb = const.tile([P, s], bf, tag="sin")     # sign-folded rope sin
    mask_sb = const.tile([P, P], bf, tag="mask")   # tril (k<=q) diagonal mask
    ones_sb = const.tile([P, P], bf, tag="ones")
    wv_sb = const.tile([P, di_t, DG], bf, tag="wv")
    wo_sb = const.tile([P, HPG, dout], bf, tag="wo")

    # persistent activation stores
    qt_sb = stores.tile([P, HPG, s], bf, tag="qt")
    kt_sb = stores.tile([P, HPG, s], bf, tag="kt")
    v_sb = stores.tile([P, st, DG], bf, tag="v")
    ctx_sb = stores.tile([P, HPG, s], bf, tag="ctx")

    # ---- phase 1: projections + RoPE ----
    with tc.tile_pool(name="xt" + sfx, bufs=1) as xtp, \
         tc.tile_pool(name="wqk" + sfx, bufs=2) as wqkp, \
         tc.tile_pool(name="p1tmp" + sfx, bufs=5) as p1tmp, \
         tc.tile_pool(name="p1sw" + sfx, bufs=5) as p1sw, \
         tc.tile_pool(name="ps_v" + sfx, bufs=4, space="PSUM") as ps_v, \
         tc.tile_pool(name="ps_qk" + sfx, bufs=4, space="PSUM") as ps_qk:
        xt_sb = xtp.tile([P, di_t, s], bf, tag="xt")
        # wv + the first xT column-group land first so V starts fast;
        # head-0 wq/wk before the later xT groups so QK can start the
        # moment V drains; the rest follows.
        for o in range(di_t):
            nc.sync.dma_start(wv_sb[:, o, :], wv[:, o, :])
            nc.sync.dma_start(xt_sb[:, o, 0:SG], xT[:, o, 0:SG])
        nc.sync.dma_start(cos_sb[:], io["cosT"][:])
        nc.sync.dma_start(sin_sb[:], io["sinT"][:])
        nc.sync.dma_start(mask_sb[:], io["mask"][:])
        nc.sync.dma_start(ones_sb[:], io["ones"][:])
        for o in range(di_t):
            nc.sync.dma_start(
                xt_sb[:, o, SG:2 * SG], xT[:, o, SG:2 * SG]
            )
        wqk_tiles = {}
        wqk_tiles[0] = (
            wqkp.tile([P, di_t, P], bf, tag="wq", name="wq_t0"),
            wqkp.tile([P, di_t, P], bf, tag="wk", name="wk_t0"),
        )
        for o in range(di_t):
            nc.sync.dma_start(wqk_tiles[0][0][:, o, :], wq[:, o, 0:P])
            nc.sync.dma_start(wqk_tiles[0][1][:, o, :], wk[:, o, 0:P])
        # later xT groups + wo on the second HWDGE ring (scalar engine)
        # so both physical DMA rings stream the input in parallel
        for o in range(di_t):
            nc.scalar.dma_start(
                xt_sb[:, o, 2 * SG:nsg * SG], xT[:, o, 2 * SG:nsg * SG]
            )
        for o in range(HPG):
            nc.scalar.dma_start(wo_sb[:, o, :], wo[:, o, :])

        # V natural layout [s_tile, DG]. First 4 si-chains run o-major in
        # parallel so early MMs only need the first xT slices (DMA-paced
        # startup); the rest run chain-at-a-time so evacuations overlap.
        pvs = [
            ps_v.tile([P, SG], f32, tag="psv", name=f"pv0_{i}")
            for i in range(4)
        ]
        for o in range(di_t):
            for q4 in range(4):
                nc.tensor.matmul(
                    pvs[q4][:, :DG],
                    lhsT=xt_sb[:, o, q4 * P:(q4 + 1) * P],
                    rhs=wv_sb[:, o, :],
                    start=(o == 0),
                    stop=(o == di_t - 1),
                )
        for q4 in range(4):
            nc.vector.tensor_copy(v_sb[:, q4, :], pvs[q4][:, :DG])
        for si in range(4, st):
            pv = ps_v.tile([P, SG], f32, tag="psv")
            for o in range(di_t):
                nc.tensor.matmul(
                    pv[:, :DG],
                    lhsT=xt_sb[:, o, si * P:(si + 1) * P],
                    rhs=wv_sb[:, o, :],
                    start=(o == 0),
                    stop=(o == di_t - 1),
                )
            nc.vector.tensor_copy(v_sb[:, si, :], pv[:, :DG])

        # Q^T, K^T with RoPE, per head. Rotate-half = partition swap via
        # two SBUF->SBUF DMAs + sign-folded sin; pipelined 2 deep so the
        # PE never waits on the ACT psum->sbuf copy or the swap DMA.
        def emit_rope(qa, qsw, dst, hh, sl):
            t1 = p1tmp.tile([P, SG], bf, tag="t1")
            nc.vector.tensor_mul(t1, qa, cos_sb[:, sl])
            t2 = p1tmp.tile([P, SG], bf, tag="t2")
            nc.vector.tensor_mul(t2, qsw, sin_sb[:, sl])
            nc.vector.tensor_add(dst[:, hh, sl], t1, t2)

        pending = []
        for h in range(HPG):
            if h in wqk_tiles:
                wq_t, wk_t = wqk_tiles[h]
            else:
                wq_t = wqkp.tile([P, di_t, P], bf, tag="wq")
                wk_t = wqkp.tile([P, di_t, P], bf, tag="wk")
                for o in range(di_t):
                    nc.sync.dma_start(wq_t[:, o, :], wq[:, o, h * P:(h + 1) * P])
                    nc.sync.dma_start(wk_t[:, o, :], wk[:, o, h * P:(h + 1) * P])

            for g in range(nsg):
                sl = slice(g * SG, (g + 1) * SG)
                for w_t, dst in ((wq_t, qt_sb), (wk_t, kt_sb)):
                    pq = ps_qk.tile([P, SG], f32, tag="ps")
                    for o in range(di_t):
                        nc.tensor.matmul(
                            pq,
                            lhsT=w_t[:, o, :],
                            rhs=xt_sb[:, o, sl],
                            start=(o == 0),
                            stop=(o == di_t - 1),
                        )
                    qa = p1tmp.tile([P, SG], bf, tag="qa")
                    nc.scalar.copy(qa, pq)
                    qsw = p1sw.tile([P, SG], bf, tag="qsw")
                    nc.sync.dma_start(qsw[0:64, :], qa[64:128, :])
                    nc.sync.dma_start(qsw[64:128, :], qa[0:64, :])
                    pending.append((qa, qsw, dst, h, sl))
                    while len(pending) > 2:
                        emit_rope(*pending.pop(0))
        while pending:
            emit_rope(*pending.pop(0))

    # ---- phase 2+3: attention interleaved with output projection ----
    # g outer so each q-group's out-proj tiles become ready early and fill
    # the PE while later q-groups' softmax runs. Diagonal k-tiles use
    # width-trimmed matmuls; softmax sums pair-tree on DVE into one tile,
    # then a single full-width ones-matmul per (h, g) lands the sum on
    # every PSUM partition so normalization needs no broadcast.
    ps_sc = tc.alloc_tile_pool(name="ps_sc" + sfx, bufs=2, space="PSUM")
    ps_main = tc.alloc_tile_pool(name="ps_main" + sfx, bufs=3, space="PSUM")
    ps_l = tc.alloc_tile_pool(name="ps_l" + sfx, bufs=1, space="PSUM")
    with tc.tile_pool(name="p2tmp" + sfx, bufs=10) as p2tmp, \
         tc.tile_pool(name="p2rb" + sfx, bufs=3) as p2rb, \
         tc.tile_pool(name="outp" + sfx, bufs=3) as outp:

        def emit_po_chain(qt, dsl):
            po = ps_main.tile([P, SG], f32, tag="ps")
            for h in range(HPG):
                nc.tensor.matmul(
                    po,
                    lhsT=ctx_sb[:, h, qt * P:(qt + 1) * P],
                    rhs=wo_sb[:, h, dsl * SG:(dsl + 1) * SG],
                    start=(h == 0),
                    stop=(h == HPG - 1),
                )
            ob = outp.tile([P, SG], f32, tag="ob")
            nc.vector.tensor_copy(ob, po)
            nc.sync.dma_start(
                io["out"][qt * P:(qt + 1) * P, dsl * SG:(dsl + 1) * SG], ob
            )

        # descending g: densest attention first; the previous group's
        # out-proj chains are emitted between each head's scores and PV,
        # exactly where the PE would otherwise stall on the exp latency.
        po_queue = []
        for g in range(nsg - 1, -1, -1):
            qsl = slice(g * SG, (g + 1) * SG)
            jf = 4 * g          # full (below-diagonal) k-tiles
            for h in range(HPG):
                pctx = ps_main.tile([P, SG], f32, tag="ps")

                # stream 1: score MMs + exp. Full tiles in pairs; the 4
                # diagonal tiles packed into two psum tiles with trimmed
                # widths (512,384 | 256,128); tails zeroed so the sum tree
                # is uniform.
                ats = []         # (tile, [(slot, qoff, w), ...])
                for j in range(0, jf, 2):
                    ps2t = ps_sc.tile([P, 2, SG], f32, tag="ps2")
                    for jj in range(2):
                        nc.tensor.matmul(
                            ps2t[:, jj, :],
                            lhsT=kt_sb[:, h, (j + jj) * P:(j + jj + 1) * P],
                            rhs=qt_sb[:, h, qsl],
                            start=True,
                            stop=True,
                        )
                    at2 = p2tmp.tile([P, 2, SG], bf, tag="at")
                    nc.scalar.activation(at2, ps2t, Exp, scale=inv_sqrt_hd)
                    ats.append((at2, [(0, 0, SG), (1, 0, SG)]))
                for pack in ((0, 1), (2, 3)):
                    psd = ps_sc.tile([P, 2, SG], f32, tag="ps2")
                    slots = []
                    for slot, r in enumerate(pack):
                        j = jf + r
                        qoff = r * P
                        w = SG - qoff
                        nc.tensor.matmul(
                            psd[:, slot, 0:w],
                            lhsT=kt_sb[:, h, j * P:(j + 1) * P],
                            rhs=qt_sb[:, h, g * SG + qoff:(g + 1) * SG],
                            start=True,
                            stop=True,
                        )
                        slots.append((slot, qoff, w))
                    dat = p2tmp.tile([P, 2, SG], bf, tag="at")
                    for slot, qoff, w in slots:
                        # write at the global q-offset so all tiles align
                        # column-wise; zero the front for the sum tree
                        nc.scalar.activation(
                            dat[:, slot, qoff:SG], psd[:, slot, 0:w], Exp,
                            scale=inv_sqrt_hd,
                        )
                        if qoff > 0:
                            nc.vector.memset(dat[:, slot, 0:qoff], 0)
                    ats.append((dat, slots))
                # mask the true diagonal 128-block of each trimmed tile
                for dat, slots in ats[-2:]:
                    for slot, qoff, w in slots:
                        nc.vector.tensor_mul(
                            dat[:, slot, qoff:qoff + P],
                            dat[:, slot, qoff:qoff + P], mask_sb
                        )

                # softmax sums (they only need the exps): DVE pair-tree
                # to one tile (diag fronts are zeroed, so full-width adds)
                dsums = []
                for at2, _slots in ats:
                    dsv = p2tmp.tile([P, SG], bf, tag="ds")
                    nc.vector.tensor_add(dsv, at2[:, 0, :], at2[:, 1, :])
                    dsums.append(dsv)
                while len(dsums) > 1:
                    nxt = []
                    for i in range(0, len(dsums) - 1, 2):
                        d2 = p2tmp.tile([P, SG], bf, tag="ds2")
                        nc.vector.tensor_add(d2, dsums[i], dsums[i + 1])
                        nxt.append(d2)
                    if len(dsums) % 2:
                        nxt.append(dsums[-1])
                    dsums = nxt

                # PE filler while ACT drains the exps: previous group's
                # out-proj chains (in-order PE queue, so fillers must
                # come BEFORE the instructions that wait on the exps)
                for _ in range(4):
                    if po_queue:
                        emit_po_chain(*po_queue.pop(0))

                # one ones-matmul -> every partition holds the sum
                psum_l = ps_l.tile([P, SG], f32, tag="l")
                nc.tensor.matmul(
                    psum_l, lhsT=ones_sb, rhs=dsums[0], start=True, stop=True
                )
                rec = p2rb.tile([P, SG], f32, tag="rec")
                nc.vector.reciprocal_approx_fast(rec, psum_l)

                # PV accumulation in j (k-tile) order
                nmm = jf + 4
                mm_i = 0
                for at2, slots in ats:
                    for slot, qoff, w in slots:
                        nc.tensor.matmul(
                            pctx[:, qoff:SG],
                            lhsT=v_sb[:, mm_i, h * P:(h + 1) * P],
                            rhs=at2[:, slot, qoff:SG],
                            start=(mm_i == 0),
                            stop=(mm_i == nmm - 1),
                        )
                        mm_i += 1
                nc.vector.tensor_mul(ctx_sb[:, h, qsl], pctx, rec)

            po_queue = [
                (qt, dsl)
                for qt in range(4 * g, 4 * (g + 1))
                for dsl in range(nos)
            ]
        while po_queue:
            emit_po_chain(*po_queue.pop(0))

    for pool in (ps_l, ps_main, ps_sc, stores, const):
        pool.release()


def build_program(cfg=None):
    import concourse.bacc as bacc
    import concourse.mybir as mybir
    import concourse.tile as tile

    cfg = cfg or {"S": S, "D": D, "DOUT": DOUT}
    bf = mybir.dt.bfloat16
    f32 = mybir.dt.float32
    nc = bacc.Bacc()
    io = {
        "xT": nc.dram_tensor("xT", [cfg["D"], cfg["S"]], bf, kind="ExternalInput"),
        "wq": nc.dram_tensor("wq", [cfg["D"], DG], bf, kind="ExternalInput"),
        "wk": nc.dram_tensor("wk", [cfg["D"], DG], bf, kind="ExternalInput"),
        "wv": nc.dram_tensor("wv", [cfg["D"], DG], bf, kind="ExternalInput"),
        "wo": nc.dram_tensor("wo", [DG, cfg["DOUT"]], bf, kind="ExternalInput"),
        "cosT": nc.dram_tensor("cosT", [P, cfg["S"]], bf, kind="ExternalInput"),
        "sinT": nc.dram_tensor("sinT", [P, cfg["S"]], bf, kind="ExternalInput"),
        "mask": nc.dram_tensor("mask", [P, P], bf, kind="ExternalInput"),
        "ones": nc.dram_tensor("ones", [P, P], bf, kind="ExternalInput"),
        "out": nc.dram_tensor(
            "out", [cfg["S"], cfg["DOUT"]], f32, kind="ExternalOutput"
        ),
    }
    with tile.TileContext(nc) as tc:
        for rep in range(cfg.get("repeat", 1)):
            _emit(tc, io, cfg, sfx=f"_r{rep}")
    nc.finalize()
    return nc


def host_constants(s=S):
    inv = 1.0 / (10000.0 ** (np.arange(0, HEAD, 2, dtype=np.float32) / HEAD))
    pos = np.arange(s, dtype=np.float32)
    ang = pos[:, None] * inv[None, :]
    ang = np.concatenate([ang, ang], axis=-1)          # (s, HEAD)
    cosT = np.cos(ang).T.astype(BF16).copy()           # (HEAD, s)
    sinT = np.sin(ang).T.astype(np.float32)
    sinT[0:64, :] *= -1.0                              # sign-folded rotate-half
    sinT = sinT.astype(BF16).copy()
    kk = np.arange(P)[:, None]
    qq = np.arange(P)[None, :]
    mask = (kk <= qq).astype(BF16)                     # (P, P) tril in [k, q]
    ones = np.ones((P, P), BF16)
    return cosT, sinT, mask, ones


def build_in_maps(x, W_query, W_key, W_value, W_out):
    cosT, sinT, mask, ones = host_constants()
    xTb = [np.ascontiguousarray(np.asarray(x[b]).T).astype(BF16) for b in range(B)]
    in_maps = []
    for core in range(N_CORES):
        b, g = divmod(core, HG)
        gsl = slice(g * DG, (g + 1) * DG)
        in_maps.append({
            "xT": xTb[b],
            "wq": np.asarray(W_query)[:, gsl].astype(BF16).copy(),
            "wk": np.asarray(W_key)[:, gsl].astype(BF16).copy(),
            "wv": np.asarray(W_value)[:, gsl].astype(BF16).copy(),
            "wo": np.asarray(W_out)[gsl, :].astype(BF16).copy(),
            "cosT": cosT, "sinT": sinT, "mask": mask, "ones": ones,
        })
    return in_maps


def gather_out(results, in_dtype=np.float32):
    out = np.zeros((B, S, DOUT), np.float32)
    for core in range(N_CORES):
        out[core // HG] += results[core]["out"]
    return out.astype(in_dtype, copy=False)


def kernel(x, W_query, W_key, W_value, W_out):
    from concourse.bass_utils import run_bass_kernel_spmd

    x = np.asarray(x)
    nc = build_program()
    in_maps = build_in_maps(x, W_query, W_key, W_value, W_out)
    res = run_bass_kernel_spmd(nc, in_maps, core_ids=list(range(N_CORES)))
    return gather_out(res.results, x.dtype)


# revision 25
# speedup vs baseline: 1.1982x; 1.0307x over previous
"""Multi-head causal attention with RoPE on 8 Trainium2 NeuronCores.

Sharding: 2 (batch) x 4 (head-groups of 4 heads). Each core computes
QKV projections, RoPE, flash-style causal attention and its slice of the
output projection for one batch and 4 heads; partial outputs are summed
on the host (row-sharded out_proj => partial-sum reduction).

Device layout choices (everything host-prepped to avoid on-device
transposes, fp32 has no DMA-transpose path):
  - x is passed pre-transposed per batch: xT [D, S] bf16
  - Q^T, K^T computed as [head_dim, S] (lhsT = W tile, rhs = xT)
  - V computed natural [S, head_dim] (lhsT = xT tile, rhs = Wv);
    emitted o-major across 4 parallel si-chains so the PE tracks the
    x DMA stream during startup (no long cold-start stall)
  - scores computed transposed [k, q]; softmax sum over k (partitions)
    via gpsimd partition_all_reduce (lands the sum on every partition,
    so normalization needs no broadcast and no PE ones-matmul)
  - RoPE rotate-half via two SBUF->SBUF partition-swap DMAs plus a
    sign-folded sin constant (no PE permutation matmul)
  - causal diagonal k-tiles use width-trimmed score/PV matmuls
    (only the q >= k columns), one 128x128 tril mask for the true
    diagonal block
  - attention g-outer with per-q-group out-proj interleaved
"""

import math
import sys

import numpy as np

try:
    import concourse.bass as bass  # noqa: F401
except Exception:
    sys.path.insert(0, "/opt/trn_rl_repo")

import ml_dtypes

P = 128
B = 2
S = 2048
D = 2048
H = 16
HEAD = 128
N_CORES = 8
HG = 4            # head groups (tensor-parallel dimension)
HPG = H // HG     # heads per group = 4
DG = HPG * HEAD   # group width = 512
SG = 512          # q-group (free dim) size
DOUT = 2048

BF16 = ml_dtypes.bfloat16


def _emit(tc, io, cfg, sfx=""):
    """Emit the per-core program. io: dict of dram APs. cfg: sizes."""
    import concourse.mybir as mybir
    nc = tc.nc
    bf = mybir.dt.bfloat16
    f32 = mybir.dt.float32
    Exp = mybir.ActivationFunctionType.Exp

    s = cfg["S"]
    d = cfg["D"]
    dout = cfg["DOUT"]
    di_t = d // P          # d_in k-tiles
    st = s // P            # seq 128-tiles
    nsg = s // SG          # seq 512-groups
    nos = dout // SG       # out column slices
    inv_sqrt_hd = 1.0 / math.sqrt(HEAD)

    xT = io["xT"].rearrange("(o p) s -> p o s", p=P)
    wq = io["wq"].rearrange("(o p) n -> p o n", p=P)
    wk = io["wk"].rearrange("(o p) n -> p o n", p=P)
    wv = io["wv"].rearrange("(o p) n -> p o n", p=P)
    wo = io["wo"].rearrange("(o p) n -> p o n", p=P)

    const = tc.alloc_tile_pool(name="const" + sfx, bufs=1)
    stores = tc.alloc_tile_pool(name="stores" + sfx, bufs=1)

    # ---- constants (tiles only; DMAs emitted inside the phase-1 stream) ----
    cos_sb = const.tile([P, s], bf, tag="cos")
    sin_sb = const.tile([P, s], bf, tag="sin")     # sign-folded rope sin
    mask_sb = const.tile([P, P], bf, tag="mask")   # tril (k<=q) diagonal mask
    ones_sb = const.tile([P, P], bf, tag="ones")
    wv_sb = const.tile([P, di_t, DG], bf, tag="wv")
    wo_sb = const.tile([P, HPG, dout], bf, tag="wo")

    # persistent activation stores
    qt_sb = stores.tile([P, HPG, s], bf, tag="qt")
    kt_sb = stores.tile([P, HPG, s], bf, tag="kt")
    v_sb = stores.tile([P, st, DG], bf, tag="v")
    ctx_sb = stores.tile([P, HPG, s], bf, tag="ctx")

    # ---- phase 1: projections + RoPE ----
    with tc.tile_pool(name="xt" + sfx, bufs=1) as xtp, \
         tc.tile_pool(name="wqk" + sfx, bufs=2) as wqkp, \
         tc.tile_pool(name="p1tmp" + sfx, bufs=5) as p1tmp, \
         tc.tile_pool(name="p1sw" + sfx, bufs=5) as p1sw, \
         tc.tile_pool(name="ps_v" + sfx, bufs=4, space="PSUM") as ps_v, \
         tc.tile_pool(name="ps_qk" + sfx, bufs=4, space="PSUM") as ps_qk:
        xt_sb = xtp.tile([P, di_t, s], bf, tag="xt")
        # x streams as full 524KB o-rows (4KB/partition lines -> near-peak
        # DMA bw) on the sync ring; weights + constants go on the second
        # HWDGE ring (scalar engine) in parallel.
        for o in range(di_t):
            nc.sync.dma_start(xt_sb[:, o, :], xT[:, o, :])
            nc.scalar.dma_start(wv_sb[:, o, :], wv[:, o, :])
        nc.scalar.dma_start(cos_sb[:], io["cosT"][:])
        nc.scalar.dma_start(sin_sb[:], io["sinT"][:])
        nc.scalar.dma_start(mask_sb[:], io["mask"][:])
        nc.scalar.dma_start(ones_sb[:], io["ones"][:])
        wqk_tiles = {}
        wqk_tiles[0] = (
            wqkp.tile([P, di_t, P], bf, tag="wq", name="wq_t0"),
            wqkp.tile([P, di_t, P], bf, tag="wk", name="wk_t0"),
        )
        for o in range(di_t):
            nc.scalar.dma_start(wqk_tiles[0][0][:, o, :], wq[:, o, 0:P])
            nc.scalar.dma_start(wqk_tiles[0][1][:, o, :], wk[:, o, 0:P])
        for o in range(HPG):
            nc.scalar.dma_start(wo_sb[:, o, :], wo[:, o, :])

        # V natural layout [s_tile, DG]. First 8 si-chains run o-major in
        # parallel (all 8 PSUM banks) so the PE tracks the x o-row DMA
        # stream during startup; the rest run chain-at-a-time so
        # evacuations overlap.
        pvs = [
            (ps_v if i < 4 else ps_qk).tile(
                [P, SG], f32, tag="psv" if i < 4 else "ps", name=f"pv0_{i}"
            )
            for i in range(8)
        ]
        for o in range(di_t):
            for q8 in range(8):
                nc.tensor.matmul(
                    pvs[q8][:, :DG],
                    lhsT=xt_sb[:, o, q8 * P:(q8 + 1) * P],
                    rhs=wv_sb[:, o, :],
                    start=(o == 0),
                    stop=(o == di_t - 1),
                )
        for q8 in range(8):
            nc.vector.tensor_copy(v_sb[:, q8, :], pvs[q8][:, :DG])
        for si in range(8, st):
            pv = ps_v.tile([P, SG], f32, tag="psv")
            for o in range(di_t):
                nc.tensor.matmul(
                    pv[:, :DG],
                    lhsT=xt_sb[:, o, si * P:(si + 1) * P],
                    rhs=wv_sb[:, o, :],
                    start=(o == 0),
                    stop=(o == di_t - 1),
                )
            nc.vector.tensor_copy(v_sb[:, si, :], pv[:, :DG])

        # Q^T, K^T with RoPE, per head. Rotate-half = partition swap via
        # two SBUF->SBUF DMAs + sign-folded sin; pipelined 2 deep so the
        # PE never waits on the ACT psum->sbuf copy or the swap DMA.
        def emit_rope(qa, qsw, dst, hh, sl):
            t1 = p1tmp.tile([P, SG], bf, tag="t1")
            nc.vector.tensor_mul(t1, qa, cos_sb[:, sl])
            t2 = p1tmp.tile([P, SG], bf, tag="t2")
            nc.vector.tensor_mul(t2, qsw, sin_sb[:, sl])
            nc.vector.tensor_add(dst[:, hh, sl], t1, t2)

        pending = []
        for h in range(HPG):
            if h in wqk_tiles:
                wq_t, wk_t = wqk_tiles[h]
            else:
                wq_t = wqkp.tile([P, di_t, P], bf, tag="wq")
                wk_t = wqkp.tile([P, di_t, P], bf, tag="wk")
                for o in range(di_t):
                    nc.scalar.dma_start(wq_t[:, o, :], wq[:, o, h * P:(h + 1) * P])
                    nc.scalar.dma_start(wk_t[:, o, :], wk[:, o, h * P:(h + 1) * P])

            for g in range(nsg):
                sl = slice(g * SG, (g + 1) * SG)
                for w_t, dst in ((wq_t, qt_sb), (wk_t, kt_sb)):
                    pq = ps_qk.tile([P, SG], f32, tag="ps")
                    for o in range(di_t):
                        nc.tensor.matmul(
                            pq,
                            lhsT=w_t[:, o, :],
                            rhs=xt_sb[:, o, sl],
                            start=(o == 0),
                            stop=(o == di_t - 1),
                        )
                    qa = p1tmp.tile([P, SG], bf, tag="qa")
                    nc.scalar.copy(qa, pq)
                    qsw = p1sw.tile([P, SG], bf, tag="qsw")
                    nc.sync.dma_start(qsw[0:64, :], qa[64:128, :])
                    nc.sync.dma_start(qsw[64:128, :], qa[0:64, :])
                    pending.append((qa, qsw, dst, h, sl))
                    while len(pending) > 2:
                        emit_rope(*pending.pop(0))
        while pending:
            emit_rope(*pending.pop(0))

    # ---- phase 2+3: attention interleaved with output projection ----
    # g outer so each q-group's out-proj tiles become ready early and fill
    # the PE while later q-groups' softmax runs. Diagonal k-tiles use
    # width-trimmed matmuls; softmax sums pair-tree on DVE into one tile,
    # then a single full-width ones-matmul per (h, g) lands the sum on
    # every PSUM partition so normalization needs no broadcast.
    ps_sc = tc.alloc_tile_pool(name="ps_sc" + sfx, bufs=2, space="PSUM")
    ps_main = tc.alloc_tile_pool(name="ps_main" + sfx, bufs=3, space="PSUM")
    ps_l = tc.alloc_tile_pool(name="ps_l" + sfx, bufs=1, space="PSUM")
    with tc.tile_pool(name="p2tmp" + sfx, bufs=10) as p2tmp, \
         tc.tile_pool(name="p2rb" + sfx, bufs=3) as p2rb, \
         tc.tile_pool(name="outp" + sfx, bufs=3) as outp:

        def emit_po_chain(qt, dsl):
            po = ps_main.tile([P, SG], f32, tag="ps")
            for h in range(HPG):
                nc.tensor.matmul(
                    po,
                    lhsT=ctx_sb[:, h, qt * P:(qt + 1) * P],
                    rhs=wo_sb[:, h, dsl * SG:(dsl + 1) * SG],
                    start=(h == 0),
                    stop=(h == HPG - 1),
                )
            ob = outp.tile([P, SG], f32, tag="ob")
            nc.vector.tensor_copy(ob, po)
            nc.sync.dma_start(
                io["out"][qt * P:(qt + 1) * P, dsl * SG:(dsl + 1) * SG], ob
            )

        # descending g: densest attention first; the previous group's
        # out-proj chains are emitted between each head's scores and PV,
        # exactly where the PE would otherwise stall on the exp latency.
        po_queue = []
        for g in range(nsg - 1, -1, -1):
            qsl = slice(g * SG, (g + 1) * SG)
            jf = 4 * g          # full (below-diagonal) k-tiles
            for h in range(HPG):
                pctx = ps_main.tile([P, SG], f32, tag="ps")

                # stream 1: score MMs + exp. Full tiles in pairs; the 4
                # diagonal tiles packed into two psum tiles with trimmed
                # widths (512,384 | 256,128); tails zeroed so the sum tree
                # is uniform.
                ats = []         # (tile, [(slot, qoff, w), ...])
                for j in range(0, jf, 2):
                    ps2t = ps_sc.tile([P, 2, SG], f32, tag="ps2")
                    for jj in range(2):
                        nc.tensor.matmul(
                            ps2t[:, jj, :],
                            lhsT=kt_sb[:, h, (j + jj) * P:(j + jj + 1) * P],
                            rhs=qt_sb[:, h, qsl],
                            start=True,
                            stop=True,
                        )
                    at2 = p2tmp.tile([P, 2, SG], bf, tag="at")
                    nc.scalar.activation(at2, ps2t, Exp, scale=inv_sqrt_hd)
                    ats.append((at2, [(0, 0, SG), (1, 0, SG)]))
                for pack in ((0, 1), (2, 3)):
                    psd = ps_sc.tile([P, 2, SG], f32, tag="ps2")
                    slots = []
                    for slot, r in enumerate(pack):
                        j = jf + r
                        qoff = r * P
                        w = SG - qoff
                        nc.tensor.matmul(
                            psd[:, slot, 0:w],
                            lhsT=kt_sb[:, h, j * P:(j + 1) * P],
                            rhs=qt_sb[:, h, g * SG + qoff:(g + 1) * SG],
                            start=True,
                            stop=True,
                        )
                        slots.append((slot, qoff, w))
                    dat = p2tmp.tile([P, 2, SG], bf, tag="at")
                    for slot, qoff, w in slots:
                        # write at the global q-offset so all tiles align
                        # column-wise; zero the front for the sum tree
                        nc.scalar.activation(
                            dat[:, slot, qoff:SG], psd[:, slot, 0:w], Exp,
                            scale=inv_sqrt_hd,
                        )
                        if qoff > 0:
                            nc.vector.memset(dat[:, slot, 0:qoff], 0)
                    ats.append((dat, slots))
                # mask the true diagonal 128-block of each trimmed tile
                for dat, slots in ats[-2:]:
                    for slot, qoff, w in slots:
                        nc.vector.tensor_mul(
                            dat[:, slot, qoff:qoff + P],
                            dat[:, slot, qoff:qoff + P], mask_sb
                        )

                # softmax sums (they only need the exps): DVE pair-tree
                # to one tile (diag fronts are zeroed, so full-width adds)
                dsums = []
                for at2, _slots in ats:
                    dsv = p2tmp.tile([P, SG], bf, tag="ds")
                    nc.vector.tensor_add(dsv, at2[:, 0, :], at2[:, 1, :])
                    dsums.append(dsv)
                while len(dsums) > 1:
                    nxt = []
                    for i in range(0, len(dsums) - 1, 2):
                        d2 = p2tmp.tile([P, SG], bf, tag="ds2")
                        nc.vector.tensor_add(d2, dsums[i], dsums[i + 1])
                        nxt.append(d2)
                    if len(dsums) % 2:
                        nxt.append(dsums[-1])
                    dsums = nxt

                # PE filler while ACT drains the exps: previous group's
                # out-proj chains (in-order PE queue, so fillers must
                # come BEFORE the instructions that wait on the exps)
                for _ in range(4):
                    if po_queue:
                        emit_po_chain(*po_queue.pop(0))

                # one ones-matmul -> every partition holds the sum
                psum_l = ps_l.tile([P, SG], f32, tag="l")
                nc.tensor.matmul(
                    psum_l, lhsT=ones_sb, rhs=dsums[0], start=True, stop=True
                )
                rec = p2rb.tile([P, SG], f32, tag="rec")
                nc.vector.reciprocal_approx_fast(rec, psum_l)

                # PV accumulation in j (k-tile) order
                nmm = jf + 4
                mm_i = 0
                for at2, slots in ats:
                    for slot, qoff, w in slots:
                        nc.tensor.matmul(
                            pctx[:, qoff:SG],
                            lhsT=v_sb[:, mm_i, h * P:(h + 1) * P],
                            rhs=at2[:, slot, qoff:SG],
                            start=(mm_i == 0),
                            stop=(mm_i == nmm - 1),
                        )
                        mm_i += 1
                nc.vector.tensor_mul(ctx_sb[:, h, qsl], pctx, rec)

            po_queue = [
                (qt, dsl)
                for qt in range(4 * g, 4 * (g + 1))
                for dsl in range(nos)
            ]
        while po_queue:
            emit_po_chain(*po_queue.pop(0))

    for pool in (ps_l, ps_main, ps_sc, stores, const):
        pool.release()


def build_program(cfg=None):
    import concourse.bacc as bacc
    import concourse.mybir as mybir
    import concourse.tile as tile

    cfg = cfg or {"S": S, "D": D, "DOUT": DOUT}
    bf = mybir.dt.bfloat16
    f32 = mybir.dt.float32
    nc = bacc.Bacc()
    io = {
        "xT": nc.dram_tensor("xT", [cfg["D"], cfg["S"]], bf, kind="ExternalInput"),
        "wq": nc.dram_tensor("wq", [cfg["D"], DG], bf, kind="ExternalInput"),
        "wk": nc.dram_tensor("wk", [cfg["D"], DG], bf, kind="ExternalInput"),
        "wv": nc.dram_tensor("wv", [cfg["D"], DG], bf, kind="ExternalInput"),
        "wo": nc.dram_tensor("wo", [DG, cfg["DOUT"]], bf, kind="ExternalInput"),
        "cosT": nc.dram_tensor("cosT", [P, cfg["S"]], bf, kind="ExternalInput"),
        "sinT": nc.dram_tensor("sinT", [P, cfg["S"]], bf, kind="ExternalInput"),
        "mask": nc.dram_tensor("mask", [P, P], bf, kind="ExternalInput"),
        "ones": nc.dram_tensor("ones", [P, P], bf, kind="ExternalInput"),
        "out": nc.dram_tensor(
            "out", [cfg["S"], cfg["DOUT"]], f32, kind="ExternalOutput"
        ),
    }
    with tile.TileContext(nc) as tc:
        for rep in range(cfg.get("repeat", 1)):
            _emit(tc, io, cfg, sfx=f"_r{rep}")
    nc.finalize()
    return nc


def host_constants(s=S):
    inv = 1.0 / (10000.0 ** (np.arange(0, HEAD, 2, dtype=np.float32) / HEAD))
    pos = np.arange(s, dtype=np.float32)
    ang = pos[:, None] * inv[None, :]
    ang = np.concatenate([ang, ang], axis=-1)          # (s, HEAD)
    cosT = np.cos(ang).T.astype(BF16).copy()           # (HEAD, s)
    sinT = np.sin(ang).T.astype(np.float32)
    sinT[0:64, :] *= -1.0                              # sign-folded rotate-half
    sinT = sinT.astype(BF16).copy()
    kk = np.arange(P)[:, None]
    qq = np.arange(P)[None, :]
    mask = (kk <= qq).astype(BF16)                     # (P, P) tril in [k, q]
    ones = np.ones((P, P), BF16)
    return cosT, sinT, mask, ones


def build_in_maps(x, W_query, W_key, W_value, W_out):
    cosT, sinT, mask, ones = host_constants()
    xTb = [np.ascontiguousarray(np.asarray(x[b]).T).astype(BF16) for b in range(B)]
    in_maps = []
    for core in range(N_CORES):
        b, g = divmod(core, HG)
        gsl = slice(g * DG, (g + 1) * DG)
        in_maps.append({
            "xT": xTb[b],
            "wq": np.asarray(W_query)[:, gsl].astype(BF16).copy(),
            "wk": np.asarray(W_key)[:, gsl].astype(BF16).copy(),
            "wv": np.asarray(W_value)[:, gsl].astype(BF16).copy(),
            "wo": np.asarray(W_out)[gsl, :].astype(BF16).copy(),
            "cosT": cosT, "sinT": sinT, "mask": mask, "ones": ones,
        })
    return in_maps


def gather_out(results, in_dtype=np.float32):
    out = np.zeros((B, S, DOUT), np.float32)
    for core in range(N_CORES):
        out[core // HG] += results[core]["out"]
    return out.astype(in_dtype, copy=False)


def kernel(x, W_query, W_key, W_value, W_out):
    from concourse.bass_utils import run_bass_kernel_spmd

    x = np.asarray(x)
    nc = build_program()
    in_maps = build_in_maps(x, W_query, W_key, W_value, W_out)
    res = run_bass_kernel_spmd(nc, in_maps, core_ids=list(range(N_CORES)))
    return gather_out(res.results, x.dtype)


# revision 29
# speedup vs baseline: 1.2397x; 1.0346x over previous
"""Multi-head causal attention with RoPE on 8 Trainium2 NeuronCores.

Sharding: 2 (batch) x 4 (head-groups of 4 heads). Each core computes
QKV projections, RoPE, flash-style causal attention and its slice of the
output projection for one batch and 4 heads; partial outputs are summed
on the host (row-sharded out_proj => partial-sum reduction).

Device layout choices (everything host-prepped to avoid on-device
transposes, fp32 has no DMA-transpose path):
  - x is passed pre-transposed per batch: xT [D, S] bf16
  - Q^T, K^T computed as [head_dim, S] (lhsT = W tile, rhs = xT)
  - V computed natural [S, head_dim] (lhsT = xT tile, rhs = Wv);
    emitted o-major across 4 parallel si-chains so the PE tracks the
    x DMA stream during startup (no long cold-start stall)
  - scores computed transposed [k, q]; softmax sum over k (partitions)
    via gpsimd partition_all_reduce (lands the sum on every partition,
    so normalization needs no broadcast and no PE ones-matmul)
  - RoPE rotate-half via two SBUF->SBUF partition-swap DMAs plus a
    sign-folded sin constant (no PE permutation matmul)
  - causal diagonal k-tiles use width-trimmed score/PV matmuls
    (only the q >= k columns), one 128x128 tril mask for the true
    diagonal block
  - attention g-outer with per-q-group out-proj interleaved
"""

import math
import sys

import numpy as np

try:
    import concourse.bass as bass  # noqa: F401
except Exception:
    sys.path.insert(0, "/opt/trn_rl_repo")

import ml_dtypes

P = 128
B = 2
S = 2048
D = 2048
H = 16
HEAD = 128
N_CORES = 8
HG = 4            # head groups (tensor-parallel dimension)
HPG = H // HG     # heads per group = 4
DG = HPG * HEAD   # group width = 512
SG = 512          # q-group (free dim) size
DOUT = 2048

BF16 = ml_dtypes.bfloat16


def _emit(tc, io, cfg, sfx=""):
    """Emit the per-core program. io: dict of dram APs. cfg: sizes."""
    import concourse.mybir as mybir
    nc = tc.nc
    bf = mybir.dt.bfloat16
    f32 = mybir.dt.float32
    Exp = mybir.ActivationFunctionType.Exp

    s = cfg["S"]
    d = cfg["D"]
    dout = cfg["DOUT"]
    di_t = d // P          # d_in k-tiles
    st = s // P            # seq 128-tiles
    nsg = s // SG          # seq 512-groups
    nos = dout // SG       # out column slices
    inv_sqrt_hd = 1.0 / math.sqrt(HEAD)

    xT = io["xT"].rearrange("(o p) s -> p o s", p=P)
    wq = io["wq"].rearrange("(o p) n -> p o n", p=P)
    wk = io["wk"].rearrange("(o p) n -> p o n", p=P)
    wv = io["wv"].rearrange("(o p) n -> p o n", p=P)
    wo = io["wo"].rearrange("(o p) n -> p o n", p=P)

    const = tc.alloc_tile_pool(name="const" + sfx, bufs=1)
    stores = tc.alloc_tile_pool(name="stores" + sfx, bufs=1)

    # ---- constants (tiles only; DMAs emitted inside the phase-1 stream) ----
    cos_sb = const.tile([P, s], bf, tag="cos")
    sin_sb = const.tile([P, s], bf, tag="sin")     # sign-folded rope sin
    mask_sb = const.tile([P, P], bf, tag="mask")   # tril (k<=q) diagonal mask
    ones_sb = const.tile([P, P], bf, tag="ones")
    wv_sb = const.tile([P, di_t, DG], bf, tag="wv")

    # persistent activation stores
    qt_sb = stores.tile([P, HPG, s], bf, tag="qt")
    kt_sb = stores.tile([P, HPG, s], bf, tag="kt")
    v_sb = stores.tile([P, st, DG], bf, tag="v")
    ctx_sb = stores.tile([P, HPG, s], bf, tag="ctx")

    # ---- phase 1: projections + RoPE ----
    with tc.tile_pool(name="xt" + sfx, bufs=1) as xtp, \
         tc.tile_pool(name="wqk" + sfx, bufs=1) as wqkp, \
         tc.tile_pool(name="p1tmp" + sfx, bufs=5) as p1tmp, \
         tc.tile_pool(name="p1sw" + sfx, bufs=5) as p1sw, \
         tc.tile_pool(name="ps_v" + sfx, bufs=4, space="PSUM") as ps_v, \
         tc.tile_pool(name="ps_qk" + sfx, bufs=4, space="PSUM") as ps_qk:
        xt_sb = xtp.tile([P, di_t, s], bf, tag="xt")
        wq_sb = wqkp.tile([P, di_t, DG], bf, tag="wq")
        wk_sb = wqkp.tile([P, di_t, DG], bf, tag="wk")
        # x streams as full 524KB o-rows (4KB/partition lines -> near-peak
        # DMA bw); weights as a handful of large DMAs interleaved so the
        # issuing-engine cost (~0.6us per dma_start) stays negligible.
        for o in range(di_t):
            nc.sync.dma_start(xt_sb[:, o, :], xT[:, o, :])
            if o % 4 == 3:
                c = o - 3
                nc.sync.dma_start(wv_sb[:, c:o + 1, :], wv[:, c:o + 1, :])
        nc.sync.dma_start(cos_sb[:], io["cosT"][:])
        nc.sync.dma_start(sin_sb[:], io["sinT"][:])
        nc.sync.dma_start(mask_sb[:], io["mask"][:])
        nc.sync.dma_start(ones_sb[:], io["ones"][:])
        nc.sync.dma_start(wq_sb[:], wq[:])
        nc.sync.dma_start(wk_sb[:], wk[:])

        # V natural layout [s_tile, DG]. First 8 si-chains run o-major in
        # parallel (all 8 PSUM banks) so the PE tracks the x o-row DMA
        # stream during startup; the rest run chain-at-a-time so
        # evacuations overlap.
        pvs = [
            (ps_v if i < 4 else ps_qk).tile(
                [P, SG], f32, tag="psv" if i < 4 else "ps", name=f"pv0_{i}"
            )
            for i in range(8)
        ]
        for o in range(di_t):
            for q8 in range(8):
                nc.tensor.matmul(
                    pvs[q8][:, :DG],
                    lhsT=xt_sb[:, o, q8 * P:(q8 + 1) * P],
                    rhs=wv_sb[:, o, :],
                    start=(o == 0),
                    stop=(o == di_t - 1),
                )
        for q8 in range(8):
            nc.vector.tensor_copy(v_sb[:, q8, :], pvs[q8][:, :DG])
        for si in range(8, st):
            pv = ps_v.tile([P, SG], f32, tag="psv")
            for o in range(di_t):
                nc.tensor.matmul(
                    pv[:, :DG],
                    lhsT=xt_sb[:, o, si * P:(si + 1) * P],
                    rhs=wv_sb[:, o, :],
                    start=(o == 0),
                    stop=(o == di_t - 1),
                )
            nc.vector.tensor_copy(v_sb[:, si, :], pv[:, :DG])

        # Q^T, K^T with RoPE, per head. Rotate-half = partition swap via
        # two SBUF->SBUF DMAs + sign-folded sin; pipelined 2 deep so the
        # PE never waits on the ACT psum->sbuf copy or the swap DMA.
        def emit_rope(qa, qsw, dst, hh, sl):
            t1 = p1tmp.tile([P, SG], bf, tag="t1")
            nc.vector.tensor_mul(t1, qa, cos_sb[:, sl])
            t2 = p1tmp.tile([P, SG], bf, tag="t2")
            nc.vector.tensor_mul(t2, qsw, sin_sb[:, sl])
            nc.vector.tensor_add(dst[:, hh, sl], t1, t2)

        pending = []
        for h in range(HPG):
            hsl = slice(h * P, (h + 1) * P)
            for g in range(nsg):
                sl = slice(g * SG, (g + 1) * SG)
                for w_t, dst in ((wq_sb, qt_sb), (wk_sb, kt_sb)):
                    pq = ps_qk.tile([P, SG], f32, tag="ps")
                    for o in range(di_t):
                        nc.tensor.matmul(
                            pq,
                            lhsT=w_t[:, o, hsl],
                            rhs=xt_sb[:, o, sl],
                            start=(o == 0),
                            stop=(o == di_t - 1),
                        )
                    qa = p1tmp.tile([P, SG], bf, tag="qa")
                    nc.scalar.copy(qa, pq)
                    qsw = p1sw.tile([P, SG], bf, tag="qsw")
                    nc.sync.dma_start(qsw[0:64, :], qa[64:128, :])
                    nc.sync.dma_start(qsw[64:128, :], qa[0:64, :])
                    pending.append((qa, qsw, dst, h, sl))
                    while len(pending) > 2:
                        emit_rope(*pending.pop(0))
        while pending:
            emit_rope(*pending.pop(0))

    # ---- phase 2+3: attention interleaved with output projection ----
    # g outer so each q-group's out-proj tiles become ready early and fill
    # the PE while later q-groups' softmax runs. Diagonal k-tiles use
    # width-trimmed matmuls; softmax sums pair-tree on DVE into one tile,
    # then a single full-width ones-matmul per (h, g) lands the sum on
    # every PSUM partition so normalization needs no broadcast.
    ps_sc = tc.alloc_tile_pool(name="ps_sc" + sfx, bufs=2, space="PSUM")
    ps_main = tc.alloc_tile_pool(name="ps_main" + sfx, bufs=3, space="PSUM")
    ps_l = tc.alloc_tile_pool(name="ps_l" + sfx, bufs=1, space="PSUM")
    with tc.tile_pool(name="p2tmp" + sfx, bufs=10) as p2tmp, \
         tc.tile_pool(name="p2rb" + sfx, bufs=3) as p2rb, \
         tc.tile_pool(name="wop" + sfx, bufs=1) as wop, \
         tc.tile_pool(name="outp" + sfx, bufs=3) as outp:
        wo_sb = wop.tile([P, HPG, dout], bf, tag="wo")
        nc.sync.dma_start(wo_sb[:], wo[:])

        def emit_po_chain(qt, dsl):
            po = ps_main.tile([P, SG], f32, tag="ps")
            for h in range(HPG):
                nc.tensor.matmul(
                    po,
                    lhsT=ctx_sb[:, h, qt * P:(qt + 1) * P],
                    rhs=wo_sb[:, h, dsl * SG:(dsl + 1) * SG],
                    start=(h == 0),
                    stop=(h == HPG - 1),
                )
            ob = outp.tile([P, SG], f32, tag="ob")
            nc.vector.tensor_copy(ob, po)
            nc.sync.dma_start(
                io["out"][qt * P:(qt + 1) * P, dsl * SG:(dsl + 1) * SG], ob
            )

        # descending g: densest attention first; the previous group's
        # out-proj chains are emitted between each head's scores and PV,
        # exactly where the PE would otherwise stall on the exp latency.
        po_queue = []
        for g in range(nsg - 1, -1, -1):
            qsl = slice(g * SG, (g + 1) * SG)
            jf = 4 * g          # full (below-diagonal) k-tiles
            for h in range(HPG):
                pctx = ps_main.tile([P, SG], f32, tag="ps")

                # stream 1: score MMs + exp. Full tiles in pairs; the 4
                # diagonal tiles packed into two psum tiles with trimmed
                # widths (512,384 | 256,128); tails zeroed so the sum tree
                # is uniform.
                ats = []         # (tile, [(slot, qoff, w), ...])
                for j in range(0, jf, 2):
                    ps2t = ps_sc.tile([P, 2, SG], f32, tag="ps2")
                    for jj in range(2):
                        nc.tensor.matmul(
                            ps2t[:, jj, :],
                            lhsT=kt_sb[:, h, (j + jj) * P:(j + jj + 1) * P],
                            rhs=qt_sb[:, h, qsl],
                            start=True,
                            stop=True,
                        )
                    at2 = p2tmp.tile([P, 2, SG], bf, tag="at")
                    nc.scalar.activation(at2, ps2t, Exp, scale=inv_sqrt_hd)
                    ats.append((at2, [(0, 0, SG), (1, 0, SG)]))
                for pack in ((0, 1), (2, 3)):
                    psd = ps_sc.tile([P, 2, SG], f32, tag="ps2")
                    slots = []
                    for slot, r in enumerate(pack):
                        j = jf + r
                        qoff = r * P
                        w = SG - qoff
                        nc.tensor.matmul(
                            psd[:, slot, 0:w],
                            lhsT=kt_sb[:, h, j * P:(j + 1) * P],
                            rhs=qt_sb[:, h, g * SG + qoff:(g + 1) * SG],
                            start=True,
                            stop=True,
                        )
                        slots.append((slot, qoff, w))
                    dat = p2tmp.tile([P, 2, SG], bf, tag="at")
                    for slot, qoff, w in slots:
                        # write at the global q-offset so all tiles align
                        # column-wise; zero the front for the sum tree
                        nc.scalar.activation(
                            dat[:, slot, qoff:SG], psd[:, slot, 0:w], Exp,
                            scale=inv_sqrt_hd,
                        )
                        if qoff > 0:
                            nc.vector.memset(dat[:, slot, 0:qoff], 0)
                    ats.append((dat, slots))
                # mask the true diagonal 128-block of each trimmed tile
                for dat, slots in ats[-2:]:
                    for slot, qoff, w in slots:
                        nc.vector.tensor_mul(
                            dat[:, slot, qoff:qoff + P],
                            dat[:, slot, qoff:qoff + P], mask_sb
                        )

                # softmax sums (they only need the exps): DVE pair-tree
                # to one tile (diag fronts are zeroed, so full-width adds)
                dsums = []
                for at2, _slots in ats:
                    dsv = p2tmp.tile([P, SG], bf, tag="ds")
                    nc.vector.tensor_add(dsv, at2[:, 0, :], at2[:, 1, :])
                    dsums.append(dsv)
                while len(dsums) > 1:
                    nxt = []
                    for i in range(0, len(dsums) - 1, 2):
                        d2 = p2tmp.tile([P, SG], bf, tag="ds2")
                        nc.vector.tensor_add(d2, dsums[i], dsums[i + 1])
                        nxt.append(d2)
                    if len(dsums) % 2:
                        nxt.append(dsums[-1])
                    dsums = nxt

                # PE filler while ACT drains the exps: previous group's
                # out-proj chains (in-order PE queue, so fillers must
                # come BEFORE the instructions that wait on the exps)
                for _ in range(4):
                    if po_queue:
                        emit_po_chain(*po_queue.pop(0))

                # one ones-matmul -> every partition holds the sum
                psum_l = ps_l.tile([P, SG], f32, tag="l")
                nc.tensor.matmul(
                    psum_l, lhsT=ones_sb, rhs=dsums[0], start=True, stop=True
                )
                rec = p2rb.tile([P, SG], f32, tag="rec")
                nc.vector.reciprocal_approx_fast(rec, psum_l)

                # PV accumulation in j (k-tile) order
                nmm = jf + 4
                mm_i = 0
                for at2, slots in ats:
                    for slot, qoff, w in slots:
                        nc.tensor.matmul(
                            pctx[:, qoff:SG],
                            lhsT=v_sb[:, mm_i, h * P:(h + 1) * P],
                            rhs=at2[:, slot, qoff:SG],
                            start=(mm_i == 0),
                            stop=(mm_i == nmm - 1),
                        )
                        mm_i += 1
                nc.vector.tensor_mul(ctx_sb[:, h, qsl], pctx, rec)

            po_queue = [
                (qt, dsl)
                for qt in range(4 * g, 4 * (g + 1))
                for dsl in range(nos)
            ]
        while po_queue:
            emit_po_chain(*po_queue.pop(0))

    for pool in (ps_l, ps_main, ps_sc, stores, const):
        pool.release()


def build_program(cfg=None):
    import concourse.bacc as bacc
    import concourse.mybir as mybir
    import concourse.tile as tile

    cfg = cfg or {"S": S, "D": D, "DOUT": DOUT}
    bf = mybir.dt.bfloat16
    f32 = mybir.dt.float32
    nc = bacc.Bacc()
    io = {
        "xT": nc.dram_tensor("xT", [cfg["D"], cfg["S"]], bf, kind="ExternalInput"),
        "wq": nc.dram_tensor("wq", [cfg["D"], DG], bf, kind="ExternalInput"),
        "wk": nc.dram_tensor("wk", [cfg["D"], DG], bf, kind="ExternalInput"),
        "wv": nc.dram_tensor("wv", [cfg["D"], DG], bf, kind="ExternalInput"),
        "wo": nc.dram_tensor("wo", [DG, cfg["DOUT"]], bf, kind="ExternalInput"),
        "cosT": nc.dram_tensor("cosT", [P, cfg["S"]], bf, kind="ExternalInput"),
        "sinT": nc.dram_tensor("sinT", [P, cfg["S"]], bf, kind="ExternalInput"),
        "mask": nc.dram_tensor("mask", [P, P], bf, kind="ExternalInput"),
        "ones": nc.dram_tensor("ones", [P, P], bf, kind="ExternalInput"),
        "out": nc.dram_tensor(
            "out", [cfg["S"], cfg["DOUT"]], f32, kind="ExternalOutput"
        ),
    }
    with tile.TileContext(nc) as tc:
        for rep in range(cfg.get("repeat", 1)):
            _emit(tc, io, cfg, sfx=f"_r{rep}")
    nc.finalize()
    return nc


def host_constants(s=S):
    inv = 1.0 / (10000.0 ** (np.arange(0, HEAD, 2, dtype=np.float32) / HEAD))
    pos = np.arange(s, dtype=np.float32)
    ang = pos[:, None] * inv[None, :]
    ang = np.concatenate([ang, ang], axis=-1)          # (s, HEAD)
    cosT = np.cos(ang).T.astype(BF16).copy()           # (HEAD, s)
    sinT = np.sin(ang).T.astype(np.float32)
    sinT[0:64, :] *= -1.0                              # sign-folded rotate-half
    sinT = sinT.astype(BF16).copy()
    kk = np.arange(P)[:, None]
    qq = np.arange(P)[None, :]
    mask = (kk <= qq).astype(BF16)                     # (P, P) tril in [k, q]
    ones = np.ones((P, P), BF16)
    return cosT, sinT, mask, ones


def build_in_maps(x, W_query, W_key, W_value, W_out):
    cosT, sinT, mask, ones = host_constants()
    xTb = [np.ascontiguousarray(np.asarray(x[b]).T).astype(BF16) for b in range(B)]
    in_maps = []
    for core in range(N_CORES):
        b, g = divmod(core, HG)
        gsl = slice(g * DG, (g + 1) * DG)
        in_maps.append({
            "xT": xTb[b],
            "wq": np.asarray(W_query)[:, gsl].astype(BF16).copy(),
            "wk": np.asarray(W_key)[:, gsl].astype(BF16).copy(),
            "wv": np.asarray(W_value)[:, gsl].astype(BF16).copy(),
            "wo": np.asarray(W_out)[gsl, :].astype(BF16).copy(),
            "cosT": cosT, "sinT": sinT, "mask": mask, "ones": ones,
        })
    return in_maps


def gather_out(results, in_dtype=np.float32):
    out = np.zeros((B, S, DOUT), np.float32)
    for core in range(N_CORES):
        out[core // HG] += results[core]["out"]
    return out.astype(in_dtype, copy=False)


def kernel(x, W_query, W_key, W_value, W_out):
    from concourse.bass_utils import run_bass_kernel_spmd

    x = np.asarray(x)
    nc = build_program()
    in_maps = build_in_maps(x, W_query, W_key, W_value, W_out)
    res = run_bass_kernel_spmd(nc, in_maps, core_ids=list(range(N_CORES)))
    return gather_out(res.results, x.dtype)


# revision 33
# speedup vs baseline: 1.2760x; 1.0293x over previous
"""Multi-head causal attention with RoPE on 8 Trainium2 NeuronCores.

Sharding: 2 (batch) x 4 (head-groups of 4 heads). Each core computes
QKV projections, RoPE, flash-style causal attention and its slice of the
output projection for one batch and 4 heads; partial outputs are summed
on the host (row-sharded out_proj => partial-sum reduction).

Device layout choices (everything host-prepped to avoid on-device
transposes, fp32 has no DMA-transpose path):
  - x is passed pre-transposed per batch: xT [D, S] bf16
  - Q^T, K^T computed as [head_dim, S] (lhsT = W tile, rhs = xT)
  - V computed natural [S, head_dim] (lhsT = xT tile, rhs = Wv);
    emitted o-major across 4 parallel si-chains so the PE tracks the
    x DMA stream during startup (no long cold-start stall)
  - scores computed transposed [k, q]; softmax sum over k (partitions)
    via gpsimd partition_all_reduce (lands the sum on every partition,
    so normalization needs no broadcast and no PE ones-matmul)
  - RoPE rotate-half via two SBUF->SBUF partition-swap DMAs plus a
    sign-folded sin constant (no PE permutation matmul)
  - causal diagonal k-tiles use width-trimmed score/PV matmuls
    (only the q >= k columns), one 128x128 tril mask for the true
    diagonal block
  - attention g-outer with per-q-group out-proj interleaved
"""

import math
import sys

import numpy as np

try:
    import concourse.bass as bass  # noqa: F401
except Exception:
    sys.path.insert(0, "/opt/trn_rl_repo")

import ml_dtypes

P = 128
B = 2
S = 2048
D = 2048
H = 16
HEAD = 128
N_CORES = 8
HG = 4            # head groups (tensor-parallel dimension)
HPG = H // HG     # heads per group = 4
DG = HPG * HEAD   # group width = 512
SG = 512          # q-group (free dim) size
DOUT = 2048

BF16 = ml_dtypes.bfloat16


def _emit(tc, io, cfg, sfx=""):
    """Emit the per-core program. io: dict of dram APs. cfg: sizes."""
    import concourse.mybir as mybir
    nc = tc.nc
    bf = mybir.dt.bfloat16
    f32 = mybir.dt.float32
    Exp = mybir.ActivationFunctionType.Exp

    s = cfg["S"]
    d = cfg["D"]
    dout = cfg["DOUT"]
    di_t = d // P          # d_in k-tiles
    st = s // P            # seq 128-tiles
    nsg = s // SG          # seq 512-groups
    nos = dout // SG       # out column slices
    inv_sqrt_hd = 1.0 / math.sqrt(HEAD)

    xT = io["xT"].rearrange("(o p) s -> p o s", p=P)
    wq = io["wq"].rearrange("(o p) n -> p o n", p=P)
    wk = io["wk"].rearrange("(o p) n -> p o n", p=P)
    wv = io["wv"].rearrange("(o p) n -> p o n", p=P)
    wo = io["wo"].rearrange("(o p) n -> p o n", p=P)

    const = tc.alloc_tile_pool(name="const" + sfx, bufs=1)
    stores = tc.alloc_tile_pool(name="stores" + sfx, bufs=1)

    # ---- constants (tiles only; DMAs emitted inside the phase-1 stream) ----
    cos_sb = const.tile([P, s], bf, tag="cos")
    sin_sb = const.tile([P, s], bf, tag="sin")     # sign-folded rope sin
    mask_sb = const.tile([P, P], bf, tag="mask")   # tril (k<=q) diagonal mask
    ones_sb = const.tile([P, P], bf, tag="ones")
    wv_sb = const.tile([P, di_t, DG], bf, tag="wv")

    # persistent activation stores
    qt_sb = stores.tile([P, HPG, s], bf, tag="qt")
    kt_sb = stores.tile([P, HPG, s], bf, tag="kt")
    v_sb = stores.tile([P, st, DG], bf, tag="v")
    ctx_sb = stores.tile([P, HPG, s], bf, tag="ctx")

    # ---- phase 1: projections + RoPE ----
    with tc.tile_pool(name="xt" + sfx, bufs=1) as xtp, \
         tc.tile_pool(name="wqk" + sfx, bufs=1) as wqkp, \
         tc.tile_pool(name="p1tmp" + sfx, bufs=5) as p1tmp, \
         tc.tile_pool(name="p1sw" + sfx, bufs=5) as p1sw, \
         tc.tile_pool(name="ps_v" + sfx, bufs=4, space="PSUM") as ps_v, \
         tc.tile_pool(name="ps_qk" + sfx, bufs=4, space="PSUM") as ps_qk:
        xt_sb = xtp.tile([P, di_t, s], bf, tag="xt")
        wq_sb = wqkp.tile([P, di_t, DG], bf, tag="wq")
        wk_sb = wqkp.tile([P, di_t, DG], bf, tag="wk")
        # x streams as full 524KB o-rows (4KB/partition lines -> near-peak
        # DMA bw); weights as a handful of large DMAs interleaved so the
        # issuing-engine cost (~0.6us per dma_start) stays negligible.
        nc.sync.dma_start(wv_sb[:, 0:4, :], wv[:, 0:4, :])
        for o in range(di_t):
            nc.sync.dma_start(xt_sb[:, o, :], xT[:, o, :])
            if o % 4 == 3 and o + 1 < di_t:
                c = o + 1
                nc.sync.dma_start(wv_sb[:, c:c + 4, :], wv[:, c:c + 4, :])
        nc.sync.dma_start(cos_sb[:], io["cosT"][:])
        nc.sync.dma_start(sin_sb[:], io["sinT"][:])
        nc.sync.dma_start(mask_sb[:], io["mask"][:])
        nc.sync.dma_start(ones_sb[:], io["ones"][:])
        nc.sync.dma_start(wq_sb[:], wq[:])
        nc.sync.dma_start(wk_sb[:], wk[:])

        # V natural layout [s_tile, DG]. First 8 si-chains run o-major in
        # parallel (all 8 PSUM banks) so the PE tracks the x o-row DMA
        # stream during startup; the rest run chain-at-a-time so
        # evacuations overlap.
        pvs = [
            (ps_v if i < 4 else ps_qk).tile(
                [P, SG], f32, tag="psv" if i < 4 else "ps", name=f"pv0_{i}"
            )
            for i in range(8)
        ]
        for o in range(di_t):
            for q8 in range(8):
                nc.tensor.matmul(
                    pvs[q8][:, :DG],
                    lhsT=xt_sb[:, o, q8 * P:(q8 + 1) * P],
                    rhs=wv_sb[:, o, :],
                    start=(o == 0),
                    stop=(o == di_t - 1),
                )
        for q8 in range(8):
            nc.vector.tensor_copy(v_sb[:, q8, :], pvs[q8][:, :DG])
        for si in range(8, st):
            pv = ps_v.tile([P, SG], f32, tag="psv")
            for o in range(di_t):
                nc.tensor.matmul(
                    pv[:, :DG],
                    lhsT=xt_sb[:, o, si * P:(si + 1) * P],
                    rhs=wv_sb[:, o, :],
                    start=(o == 0),
                    stop=(o == di_t - 1),
                )
            nc.vector.tensor_copy(v_sb[:, si, :], pv[:, :DG])

        # Q^T, K^T with RoPE, per head. Rotate-half = partition swap via
        # two SBUF->SBUF DMAs + sign-folded sin; pipelined 2 deep so the
        # PE never waits on the ACT psum->sbuf copy or the swap DMA.
        def emit_rope(qa, qsw, dst, hh, sl):
            t1 = p1tmp.tile([P, SG], bf, tag="t1")
            nc.vector.tensor_mul(t1, qa, cos_sb[:, sl])
            t2 = p1tmp.tile([P, SG], bf, tag="t2")
            nc.vector.tensor_mul(t2, qsw, sin_sb[:, sl])
            nc.vector.tensor_add(dst[:, hh, sl], t1, t2)

        pending = []
        for h in range(HPG):
            hsl = slice(h * P, (h + 1) * P)
            for g in range(nsg):
                sl = slice(g * SG, (g + 1) * SG)
                for w_t, dst in ((wq_sb, qt_sb), (wk_sb, kt_sb)):
                    pq = ps_qk.tile([P, SG], f32, tag="ps")
                    for o in range(di_t):
                        nc.tensor.matmul(
                            pq,
                            lhsT=w_t[:, o, hsl],
                            rhs=xt_sb[:, o, sl],
                            start=(o == 0),
                            stop=(o == di_t - 1),
                        )
                    qa = p1tmp.tile([P, SG], bf, tag="qa")
                    nc.scalar.copy(qa, pq)
                    qsw = p1sw.tile([P, SG], bf, tag="qsw")
                    nc.sync.dma_start(qsw[0:64, :], qa[64:128, :])
                    nc.sync.dma_start(qsw[64:128, :], qa[0:64, :])
                    pending.append((qa, qsw, dst, h, sl))
                    while len(pending) > 2:
                        emit_rope(*pending.pop(0))
        while pending:
            emit_rope(*pending.pop(0))

    # ---- phase 2+3: attention interleaved with output projection ----
    # g outer so each q-group's out-proj tiles become ready early and fill
    # the PE while later q-groups' softmax runs. Diagonal k-tiles use
    # width-trimmed matmuls; softmax sums pair-tree on DVE into one tile,
    # then a single full-width ones-matmul per (h, g) lands the sum on
    # every PSUM partition so normalization needs no broadcast.
    ps_sc = tc.alloc_tile_pool(name="ps_sc" + sfx, bufs=2, space="PSUM")
    ps_main = tc.alloc_tile_pool(name="ps_main" + sfx, bufs=3, space="PSUM")
    ps_l = tc.alloc_tile_pool(name="ps_l" + sfx, bufs=1, space="PSUM")
    with tc.tile_pool(name="p2tmp" + sfx, bufs=10) as p2tmp, \
         tc.tile_pool(name="p2rb" + sfx, bufs=3) as p2rb, \
         tc.tile_pool(name="wop" + sfx, bufs=1) as wop, \
         tc.tile_pool(name="outp" + sfx, bufs=3) as outp:
        wo_sb = wop.tile([P, HPG, dout], bf, tag="wo")
        nc.sync.dma_start(wo_sb[:], wo[:])

        def emit_po_chain(qt, dsl, dma_eng=None):
            po = ps_main.tile([P, SG], f32, tag="ps")
            for h in range(HPG):
                nc.tensor.matmul(
                    po,
                    lhsT=ctx_sb[:, h, qt * P:(qt + 1) * P],
                    rhs=wo_sb[:, h, dsl * SG:(dsl + 1) * SG],
                    start=(h == 0),
                    stop=(h == HPG - 1),
                )
            ob = outp.tile([P, SG], f32, tag="ob")
            nc.vector.tensor_copy(ob, po)
            (dma_eng or nc.sync).dma_start(
                io["out"][qt * P:(qt + 1) * P, dsl * SG:(dsl + 1) * SG], ob
            )

        # descending g: densest attention first; the previous group's
        # out-proj chains are emitted between each head's scores and PV,
        # exactly where the PE would otherwise stall on the exp latency.
        po_queue = []
        for g in range(nsg - 1, -1, -1):
            qsl = slice(g * SG, (g + 1) * SG)
            jf = 4 * g          # full (below-diagonal) k-tiles
            for h in range(HPG):
                pctx = ps_main.tile([P, SG], f32, tag="ps")

                # stream 1: score MMs + exp. Full tiles in pairs; the 4
                # diagonal tiles packed into two psum tiles with trimmed
                # widths (512,384 | 256,128); tails zeroed so the sum tree
                # is uniform.
                ats = []         # (tile, [(slot, qoff, w), ...])
                for j in range(0, jf, 2):
                    ps2t = ps_sc.tile([P, 2, SG], f32, tag="ps2")
                    for jj in range(2):
                        nc.tensor.matmul(
                            ps2t[:, jj, :],
                            lhsT=kt_sb[:, h, (j + jj) * P:(j + jj + 1) * P],
                            rhs=qt_sb[:, h, qsl],
                            start=True,
                            stop=True,
                        )
                    at2 = p2tmp.tile([P, 2, SG], bf, tag="at")
                    nc.scalar.activation(at2, ps2t, Exp, scale=inv_sqrt_hd)
                    ats.append((at2, [(0, 0, SG), (1, 0, SG)]))
                for pack in ((0, 1), (2, 3)):
                    psd = ps_sc.tile([P, 2, SG], f32, tag="ps2")
                    slots = []
                    for slot, r in enumerate(pack):
                        j = jf + r
                        qoff = r * P
                        w = SG - qoff
                        nc.tensor.matmul(
                            psd[:, slot, 0:w],
                            lhsT=kt_sb[:, h, j * P:(j + 1) * P],
                            rhs=qt_sb[:, h, g * SG + qoff:(g + 1) * SG],
                            start=True,
                            stop=True,
                        )
                        slots.append((slot, qoff, w))
                    dat = p2tmp.tile([P, 2, SG], bf, tag="at")
                    for slot, qoff, w in slots:
                        # write at the global q-offset so all tiles align
                        # column-wise; zero the front for the sum tree
                        nc.scalar.activation(
                            dat[:, slot, qoff:SG], psd[:, slot, 0:w], Exp,
                            scale=inv_sqrt_hd,
                        )
                        if qoff > 0:
                            nc.vector.memset(dat[:, slot, 0:qoff], 0)
                    ats.append((dat, slots))
                # mask the true diagonal 128-block of each trimmed tile
                for dat, slots in ats[-2:]:
                    for slot, qoff, w in slots:
                        nc.vector.tensor_mul(
                            dat[:, slot, qoff:qoff + P],
                            dat[:, slot, qoff:qoff + P], mask_sb
                        )

                # softmax sums (they only need the exps): DVE pair-tree
                # to one tile (diag fronts are zeroed, so full-width adds)
                dsums = []
                for at2, _slots in ats:
                    dsv = p2tmp.tile([P, SG], bf, tag="ds")
                    nc.vector.tensor_add(dsv, at2[:, 0, :], at2[:, 1, :])
                    dsums.append(dsv)
                while len(dsums) > 1:
                    nxt = []
                    for i in range(0, len(dsums) - 1, 2):
                        d2 = p2tmp.tile([P, SG], bf, tag="ds2")
                        nc.vector.tensor_add(d2, dsums[i], dsums[i + 1])
                        nxt.append(d2)
                    if len(dsums) % 2:
                        nxt.append(dsums[-1])
                    dsums = nxt

                # PE filler while ACT drains the exps: previous group's
                # out-proj chains (in-order PE queue, so fillers must
                # come BEFORE the instructions that wait on the exps)
                for _ in range(4):
                    if po_queue:
                        emit_po_chain(*po_queue.pop(0))

                # one ones-matmul -> every partition holds the sum
                psum_l = ps_l.tile([P, SG], f32, tag="l")
                nc.tensor.matmul(
                    psum_l, lhsT=ones_sb, rhs=dsums[0], start=True, stop=True
                )
                rec = p2rb.tile([P, SG], f32, tag="rec")
                nc.vector.reciprocal_approx_fast(rec, psum_l)

                # PV accumulation in j (k-tile) order
                nmm = jf + 4
                mm_i = 0
                for at2, slots in ats:
                    for slot, qoff, w in slots:
                        nc.tensor.matmul(
                            pctx[:, qoff:SG],
                            lhsT=v_sb[:, mm_i, h * P:(h + 1) * P],
                            rhs=at2[:, slot, qoff:SG],
                            start=(mm_i == 0),
                            stop=(mm_i == nmm - 1),
                        )
                        mm_i += 1
                nc.vector.tensor_mul(ctx_sb[:, h, qsl], pctx, rec)

            po_queue = [
                (qt, dsl)
                for qt in range(4 * g, 4 * (g + 1))
                for dsl in range(nos)
            ]
        # final drain: alternate output DMAs across both HWDGE rings
        # (the scalar ring is idle once the last exps are done)
        flip = False
        while po_queue:
            emit_po_chain(*po_queue.pop(0), dma_eng=nc.scalar if flip else nc.sync)
            flip = not flip

    for pool in (ps_l, ps_main, ps_sc, stores, const):
        pool.release()


def build_program(cfg=None):
    import concourse.bacc as bacc
    import concourse.mybir as mybir
    import concourse.tile as tile

    cfg = cfg or {"S": S, "D": D, "DOUT": DOUT}
    bf = mybir.dt.bfloat16
    f32 = mybir.dt.float32
    nc = bacc.Bacc()
    io = {
        "xT": nc.dram_tensor("xT", [cfg["D"], cfg["S"]], bf, kind="ExternalInput"),
        "wq": nc.dram_tensor("wq", [cfg["D"], DG], bf, kind="ExternalInput"),
        "wk": nc.dram_tensor("wk", [cfg["D"], DG], bf, kind="ExternalInput"),
        "wv": nc.dram_tensor("wv", [cfg["D"], DG], bf, kind="ExternalInput"),
        "wo": nc.dram_tensor("wo", [DG, cfg["DOUT"]], bf, kind="ExternalInput"),
        "cosT": nc.dram_tensor("cosT", [P, cfg["S"]], bf, kind="ExternalInput"),
        "sinT": nc.dram_tensor("sinT", [P, cfg["S"]], bf, kind="ExternalInput"),
        "mask": nc.dram_tensor("mask", [P, P], bf, kind="ExternalInput"),
        "ones": nc.dram_tensor("ones", [P, P], bf, kind="ExternalInput"),
        "out": nc.dram_tensor(
            "out", [cfg["S"], cfg["DOUT"]], f32, kind="ExternalOutput"
        ),
    }
    with tile.TileContext(nc) as tc:
        for rep in range(cfg.get("repeat", 1)):
            _emit(tc, io, cfg, sfx=f"_r{rep}")
    nc.finalize()
    return nc


def host_constants(s=S):
    inv = 1.0 / (10000.0 ** (np.arange(0, HEAD, 2, dtype=np.float32) / HEAD))
    pos = np.arange(s, dtype=np.float32)
    ang = pos[:, None] * inv[None, :]
    ang = np.concatenate([ang, ang], axis=-1)          # (s, HEAD)
    cosT = np.cos(ang).T.astype(BF16).copy()           # (HEAD, s)
    sinT = np.sin(ang).T.astype(np.float32)
    sinT[0:64, :] *= -1.0                              # sign-folded rotate-half
    sinT = sinT.astype(BF16).copy()
    kk = np.arange(P)[:, None]
    qq = np.arange(P)[None, :]
    mask = (kk <= qq).astype(BF16)                     # (P, P) tril in [k, q]
    ones = np.ones((P, P), BF16)
    return cosT, sinT, mask, ones


def build_in_maps(x, W_query, W_key, W_value, W_out):
    cosT, sinT, mask, ones = host_constants()
    xTb = [np.ascontiguousarray(np.asarray(x[b]).T).astype(BF16) for b in range(B)]
    in_maps = []
    for core in range(N_CORES):
        b, g = divmod(core, HG)
        gsl = slice(g * DG, (g + 1) * DG)
        in_maps.append({
            "xT": xTb[b],
            "wq": np.asarray(W_query)[:, gsl].astype(BF16).copy(),
            "wk": np.asarray(W_key)[:, gsl].astype(BF16).copy(),
            "wv": np.asarray(W_value)[:, gsl].astype(BF16).copy(),
            "wo": np.asarray(W_out)[gsl, :].astype(BF16).copy(),
            "cosT": cosT, "sinT": sinT, "mask": mask, "ones": ones,
        })
    return in_maps


def gather_out(results, in_dtype=np.float32):
    out = np.zeros((B, S, DOUT), np.float32)
    for core in range(N_CORES):
        out[core // HG] += results[core]["out"]
    return out.astype(in_dtype, copy=False)


def kernel(x, W_query, W_key, W_value, W_out):
    from concourse.bass_utils import run_bass_kernel_spmd

    x = np.asarray(x)
    nc = build_program()
    in_maps = build_in_maps(x, W_query, W_key, W_value, W_out)
    res = run_bass_kernel_spmd(nc, in_maps, core_ids=list(range(N_CORES)))
    return gather_out(res.results, x.dtype)
